# revision 24
# baseline (speedup 1.0000x reference)
"""Trainium2 Bass kernel for nn_NodeNet (GNN message passing + 15-qubit circuit).

Exact algebraic structure exploited (hand-scheduled version):
1. The 2^15 state stays a tensor product of small components; only the
   q5 component reaches 256 dims and only it depends on the message-passing
   matmuls.  The q10 measurement chain depends on X only, so it runs
   entirely inside the input-DMA window.
2. Final CNOT+RY before each measurement are folded into the observable
   (Heisenberg picture): O = cos(th)*Z_c Z_t + sin(th)*X_t, measured with
   3 fused multiply-accumulate ops instead of gate applications.
3. The last CNOT(3,7)+RY(b7) pair on the q5 chain folds into the m5 x m6
   merge by pre-rotating m6 two ways (theta20+-alpha) - the 256-wide RY
   disappears.
4. Range reduction for sin/cos uses mod: sin(u/2) = Sin(pi - 2pi*frac(
   u/(4pi)+16)), one fused ACT op produces both sin and cos columns.
5. DMA plan minimizes HWDGE serialization (a single device in HW): big
   matrices split between the SP HWDGE queue and the Pool SWDGE queue,
   the last-arriving piece is only 128 columns wide.

Self-contained: hardcodes shapes (N=128, E=1024) and the (pre-simplified)
gate structure.
"""

import math

import numpy as np

N_CORES = 8
PI = math.pi

_cache = {}


def _build_program():
    import concourse.bacc as bacc
    import concourse.mybir as mybir
    import concourse.tile as tile
    from concourse.masks import make_identity

    f32 = mybir.dt.float32
    i32 = mybir.dt.int32
    Alu = mybir.AluOpType
    Act = mybir.ActivationFunctionType

    nc = bacc.Bacc(
        "TRN2",
        target_bir_lowering=False,
        debug=False,
        enable_asserts=False,
        num_devices=1,
    )

    X_d = nc.dram_tensor("X", [128, 5], f32, kind="ExternalInput").ap()
    e_d = nc.dram_tensor("e", [1024], f32, kind="ExternalInput").ap()
    Ri_d = nc.dram_tensor("Ri", [128, 1024], f32, kind="ExternalInput").ap()
    Ro_d = nc.dram_tensor("Ro", [128, 1024], f32, kind="ExternalInput").ap()
    th_d = nc.dram_tensor("theta", [31], f32, kind="ExternalInput").ap()
    out_d = nc.dram_tensor("out", [128, 2], f32, kind="ExternalOutput").ap()

    with tile.TileContext(nc) as tc:
        with (
            tc.tile_pool(name="sbuf", bufs=1) as sb,
            tc.tile_pool(name="psmm", bufs=1, space="PSUM") as psmm,
            tc.tile_pool(name="pstp", bufs=1, space="PSUM") as pstp,
            tc.tile_pool(name="psac", bufs=1, space="PSUM") as psac,
            tc.tile_pool(name="psac2", bufs=1, space="PSUM") as psac2,
        ):
            # ---------------- SBUF tiles ----------------
            Ri_sb = sb.tile([128, 1024], f32, tag="Ri")
            Ro_sb = sb.tile([128, 1024], f32, tag="Ro")
            RiT = sb.tile([128, 1024], f32, tag="RiT")
            RoT = sb.tile([128, 1024], f32, tag="RoT")
            X_sb = sb.tile([128, 5], f32, tag="X")
            TH = sb.tile([128, 31], f32, tag="TH")
            e8_sb = sb.tile([8, 128], f32, tag="e8")
            e_sb = sb.tile([128, 8], f32, tag="e_sb")
            ident = sb.tile([128, 128], f32, tag="ident")

            # ---------------- DMA dispatches ----------------
            # SP HWDGE queue (in order): Ri halves first (earliest big
            # transfers), then theta (broadcast to 128 partitions), e8,
            # and the tail 384 cols of Ro.
            th1 = sb.tile([1, 31], f32, tag="th1")
            nc.sync.dma_start(th1[:], th_d.unsqueeze(0))
            nc.sync.dma_start(Ri_sb[:, 0:512], Ri_d[:, 0:512])
            nc.sync.dma_start(Ri_sb[:, 512:1024], Ri_d[:, 512:1024])
            nc.sync.dma_start(e8_sb[:], e_d.rearrange("(c p) -> c p", c=8))
            nc.sync.dma_start(Ro_sb[:, 640:1024], Ro_d[:, 640:1024])

            # Pool SWDGE queue: X, then Ro cols 0-511, then the small
            # 128-col piece (arrives last; short matmul tail).
            nc.gpsimd.dma_start(X_sb[:], X_d)
            make_identity(nc, ident[:])
            nc.gpsimd.dma_start(Ro_sb[:, 0:512], Ro_d[:, 0:512])
            nc.gpsimd.dma_start(Ro_sb[:, 512:640], Ro_d[:, 512:640])

            # theta broadcast to all partitions via K=1 matmul
            ones1 = sb.tile([1, 128], f32, tag="ones1")
            nc.vector.memset(ones1[:], 1.0)
            th_ps = psac.tile([128, 31], f32, tag="th_ps")
            nc.tensor.matmul(th_ps[:], ones1[:], th1[:], start=True, stop=True)
            nc.vector.tensor_copy(TH[:], th_ps[:])

            # ---------------- constants / warmup ----------------
            # Preload the Sin activation table during the DMA window.
            warm = sb.tile([1, 1], f32, tag="warm")
            nc.vector.memset(warm[:], 0.0)
            nc.scalar.activation(warm[:], warm[:], Act.Sin)

            # sign row sigma[x] = (-1)^x, replicated across partitions
            sigI = sb.tile([128, 128], i32, tag="sigI")
            sigF = sb.tile([128, 128], f32, tag="sigF")
            nc.gpsimd.iota(sigI[:], pattern=[[1, 128]], base=0,
                           channel_multiplier=0)
            nc.vector.tensor_scalar(sigI[:], sigI[:], 1, None, Alu.bitwise_and)
            nc.vector.tensor_copy(sigF[:], sigI[:])
            nc.vector.tensor_scalar(sigF[:], sigF[:], -2.0, 1.0,
                                    Alu.mult, Alu.add)

            # ---------------- angle block A (theta + X qubits) ----------
            # AANG columns (full angles u; we produce cos/sin of u/2):
            #  0: th14   1: th15   2: th16   3: th19   4: th25
            #  5: th17+th21        6: th24+th27
            #  7: th20+th23+th26   8: th20-th23-th26
            #  9: 2*th29          10: 2*th30
            # 11: X0+th10  12: X1+th11  13: X3+th13+th18+th22
            # 14: X4+th14+th19+th28    15: 2*col14
            _hp = tc.high_priority()
            _hp.__enter__()
            AANG = sb.tile([128, 16], f32, tag="AANG")
            scr2 = sb.tile([128, 2], f32, tag="scr2")

            THv = TH[:].rearrange("p (o x) -> p o x", o=1)

            def thcols(lo, n, step=1):
                return THv[:, :, lo:lo + (n - 1) * step + 1:step] if step > 1 \
                    else TH[:, lo:lo + n]

            AAv = AANG[:].rearrange("p (o x) -> p o x", o=1)

            nc.vector.tensor_copy(AANG[:, 0:3], TH[:, 14:17])
            nc.vector.tensor_copy(AAv[:, :, 3:5], thcols(19, 2, 6))
            nc.vector.tensor_tensor(AAv[:, :, 5:7], thcols(17, 2, 7),
                                    thcols(21, 2, 6), Alu.add)
            nc.vector.tensor_tensor(scr2[:, 0:1], TH[:, 23:24], TH[:, 26:27],
                                    Alu.add)
            nc.vector.tensor_tensor(AANG[:, 7:8], TH[:, 20:21], scr2[:, 0:1],
                                    Alu.add)
            nc.vector.tensor_tensor(AANG[:, 8:9], TH[:, 20:21], scr2[:, 0:1],
                                    Alu.subtract)
            nc.vector.tensor_tensor(AANG[:, 9:11], TH[:, 29:31], TH[:, 29:31],
                                    Alu.add)
            nc.vector.tensor_tensor(AANG[:, 11:13], X_sb[:, 0:2], TH[:, 10:12],
                                    Alu.add)
            nc.vector.tensor_tensor(scr2[:], TH[:, 13:15], TH[:, 18:20],
                                    Alu.add)
            nc.vector.tensor_tensor(scr2[:].rearrange("p (o x) -> p o x", o=1),
                                    scr2[:].rearrange("p (o x) -> p o x", o=1),
                                    thcols(22, 2, 6), Alu.add)
            nc.vector.tensor_tensor(AANG[:, 13:15], X_sb[:, 3:5], scr2[:],
                                    Alu.add)
            nc.vector.tensor_tensor(AANG[:, 15:16], AANG[:, 14:15],
                                    AANG[:, 14:15], Alu.add)

            # sincos A: csA[:, j] = sin(u_j/2), csA[:, 16+j] = cos(u_j/2)
            wsA = sb.tile([128, 32], f32, tag="wsA")
            csA = sb.tile([128, 32], f32, tag="csA")
            K4 = 1.0 / (4.0 * PI)
            nc.vector.tensor_scalar(wsA[:, 0:16], AANG[:], K4, 16.0,
                                    Alu.mult, Alu.add)
            nc.vector.tensor_scalar(wsA[:, 16:32], AANG[:], K4, 16.25,
                                    Alu.mult, Alu.add)
            kiA = sb.tile([128, 32], i32, tag="kiA")
            kfA = sb.tile([128, 32], f32, tag="kfA")
            nc.vector.tensor_copy(kiA[:], wsA[:])
            nc.vector.tensor_copy(kfA[:], kiA[:])
            nc.vector.tensor_tensor(wsA[:], wsA[:], kfA[:], Alu.subtract)
            nc.vector.tensor_scalar(kfA[:], wsA[:], 0.5, None, Alu.is_gt)
            nc.vector.scalar_tensor_tensor(wsA[:], kfA[:], -1.0, wsA[:],
                                           Alu.mult, Alu.add)
            nc.scalar.activation(csA[:], wsA[:], Act.Sin, scale=2.0 * PI)

            def sA(j):
                return csA[:, j:j + 1]

            def cA(j):
                return csA[:, 16 + j:16 + j + 1]

            # negated sins for the folded measurements
            nsA = sb.tile([128, 2], f32, tag="nsA")
            nc.vector.tensor_scalar(nsA[:], csA[:, 9:11], -1.0, None, Alu.mult)
            _hp.__exit__(None, None, None)

            # measurement rows (ready early): Rz = cos(th29)*sigma,
            # Rzneg = -Rz
            Rz = sb.tile([128, 128], f32, tag="Rz")
            Rzn = sb.tile([128, 128], f32, tag="Rzn")
            nc.vector.tensor_scalar(Rz[:], sigF[:], cA(9), None, Alu.mult)
            nc.vector.tensor_scalar(Rzn[:], Rz[:], -1.0, None, Alu.mult)

            # ---------------- q10 chain (X-only, hidden under DMA) -------
            # m4 = merge+cnot(q11(L), q10(H)); RY(b1, col5); m8 = merge+
            # cnot(q13(L), m4(H)); RY(b2, col6); measure folded:
            # z10 = c30*cos(q14_full)*<Z_b2> + 2*s30*<X_b2-pair>
            m4 = sb.tile([128, 4], f32, tag="m4")
            m8 = sb.tile([128, 8], f32, tag="m8")
            d4 = sb.tile([128, 4], f32, tag="d4")
            d8 = sb.tile([128, 8], f32, tag="d8")
            zac = sb.tile([128, 8], f32, tag="zac")
            out_sb = sb.tile([128, 2], f32, tag="out")

            csAv = csA[:].rearrange("p (c d b) -> p c d b", d=2, b=8)
            # L=q11 pair: cols (16+12, 12); H=q10 pair: (16+11, 11)
            m4v = m4[:].rearrange("p (t c) -> p t c", c=2)
            # H[t]: t=0 -> col 27, t=1 -> col 11  (start 27 stride -16)
            Hq10 = csA[:].rearrange("p (c x) -> p c x", c=2)[:, ::-1, 11:12]
            Hq10r = csA[:].rearrange("p (c x) -> p c x", c=2)[:, :, 11:12]
            nc.vector.tensor_tensor(
                m4v[:, :, 0:1], Hq10,
                cA(12).unsqueeze(1).to_broadcast((128, 2, 1)), Alu.mult)
            nc.vector.tensor_tensor(
                m4v[:, :, 1:2], Hq10r,
                sA(12).unsqueeze(1).to_broadcast((128, 2, 1)), Alu.mult)
            # RY(m4, b1, col5): a0=m4[:,0:2] (t=0), a1=m4[:,2:4]
            nc.vector.tensor_scalar(d4[:], m4[:], sA(5), None, Alu.mult)
            nc.vector.scalar_tensor_tensor(
                m4[:, 0:2], m4[:, 0:2], cA(5), d4[:, 2:4],
                Alu.mult, Alu.subtract)
            nc.vector.scalar_tensor_tensor(
                m4[:, 2:4], m4[:, 2:4], cA(5), d4[:, 0:2],
                Alu.mult, Alu.add)
            # m8 = merge+cnot(q13(L,b0), m4(H,b1-b2)); ctrl b0, tgt b2
            m8v = m8[:].rearrange("p (h c) -> p h c", c=2)
            m4f = m4[:].rearrange("p (t i) -> p t i", t=2)
            nc.vector.tensor_tensor(
                m8v[:, :, 0:1], m4[:].unsqueeze(2),
                cA(13).unsqueeze(1).to_broadcast((128, 4, 1)), Alu.mult)
            m8v2 = m8[:].rearrange("p (hb1 hb0 c) -> p hb1 hb0 c",
                                   hb0=2, c=2)
            nc.vector.tensor_tensor(
                m8v2[:, :, :, 1:2],
                m4f[:, ::-1, :].unsqueeze(3),
                sA(13).unsqueeze(1).unsqueeze(1)
                .to_broadcast((128, 2, 2, 1)), Alu.mult)
            # RY(m8, b2, col6): a0 = cols 0-3, a1 = cols 4-7
            nc.vector.tensor_scalar(d8[:], m8[:], sA(6), None, Alu.mult)
            nc.vector.scalar_tensor_tensor(
                m8[:, 0:4], m8[:, 0:4], cA(6), d8[:, 4:8],
                Alu.mult, Alu.subtract)
            nc.vector.scalar_tensor_tensor(
                m8[:, 4:8], m8[:, 4:8], cA(6), d8[:, 0:4],
                Alu.mult, Alu.add)
            # measure: w0=sum(a0^2), w1=sum(a1^2), w2=sum(a0*a1)
            nc.vector.scalar_tensor_tensor(d8[:, 0:4], m8[:, 0:4], 1.0,
                                           m8[:, 0:4], Alu.mult, Alu.mult,
                                           accum_out=zac[:, 0:1])
            nc.vector.scalar_tensor_tensor(d8[:, 4:8], m8[:, 4:8], 1.0,
                                           m8[:, 4:8], Alu.mult, Alu.mult,
                                           accum_out=zac[:, 1:2])
            nc.vector.scalar_tensor_tensor(d4[:, 0:4], m8[:, 0:4], 1.0,
                                           m8[:, 4:8], Alu.mult, Alu.mult,
                                           accum_out=zac[:, 2:3])
            # z10 = cA(10)*cA(15)*(w0-w1) - 2*sA(10)*w2
            nc.vector.tensor_tensor(zac[:, 3:4], zac[:, 0:1], zac[:, 1:2],
                                    Alu.subtract)
            nc.vector.tensor_scalar(zac[:, 3:4], zac[:, 3:4], cA(15), cA(10),
                                    Alu.mult, Alu.mult)
            nc.vector.tensor_tensor(zac[:, 4:5], zac[:, 2:3], zac[:, 2:3],
                                    Alu.add)
            nc.vector.scalar_tensor_tensor(
                zac[:, 4:5], zac[:, 4:5], nsA[:, 1:2], zac[:, 3:4],
                Alu.mult, Alu.add)
            nc.vector.tensor_scalar(out_sb[:, 1:2], zac[:, 4:5], -PI, PI,
                                    Alu.mult, Alu.add)

            # ---------------- message passing pipeline ----------------
            # e transpose: [8,128] -> [128,8]
            e_ps = psmm.tile([128, 8], f32, tag="e_ps")
            nc.tensor.transpose(e_ps[:], e8_sb[:], ident[0:8, 0:8])
            nc.scalar.copy(e_sb[:], e_ps[:])

            # bo/bi: bb_ps[:, c*10:+5] = Ro_c^T X ; +5:+10 = Ri_c^T X
            bb_ps = psmm.tile([128, 80], f32, tag="bb")
            for c in range(8):
                nc.tensor.matmul(bb_ps[:, c * 10 + 5:c * 10 + 10],
                                 Ri_sb[:, c * 128:(c + 1) * 128], X_sb[:],
                                 start=True, stop=True)
            for c in range(8):
                nc.tensor.matmul(bb_ps[:, c * 10:c * 10 + 5],
                                 Ro_sb[:, c * 128:(c + 1) * 128], X_sb[:],
                                 start=True, stop=True)

            # transposes: Ri chunks then Ro chunks; copies alternate
            # DVE / ACT / Pool.
            def copy_out(dst, src, eng):
                if eng == 0:
                    nc.vector.tensor_copy(dst, src)
                elif eng == 1:
                    nc.scalar.copy(dst, src)
                else:
                    nc.gpsimd.tensor_copy(dst, src)

            tpri_tiles = {}
            for h in range(2):
                tp = pstp.tile([128, 512], f32, tag=f"tpri{h}")
                tpri_tiles[h] = tp
                for cc in range(4):
                    c = h * 4 + cc
                    nc.tensor.transpose(tp[:, cc * 128:(cc + 1) * 128],
                                        Ri_sb[:, c * 128:(c + 1) * 128],
                                        ident[:])
                copy_out(RiT[:, h * 512:h * 512 + 256], tp[:, 0:256], 0)
                copy_out(RiT[:, h * 512 + 256:h * 512 + 512], tp[:, 256:512],
                         0)
            for h in range(2):
                tp = tpri_tiles[0] if h == 1 else \
                    pstp.tile([128, 512], f32, tag="tpro0")
                for cc in range(4):
                    c = h * 4 + cc
                    nc.tensor.transpose(tp[:, cc * 128:(cc + 1) * 128],
                                        Ro_sb[:, c * 128:(c + 1) * 128],
                                        ident[:])
                if h == 0:
                    copy_out(RoT[:, 0:256], tp[:, 0:256], 0)
                    copy_out(RoT[:, 256:512], tp[:, 256:512], 0)
                else:
                    # last piece 512:640 arrives latest - copy its
                    # transpose separately on DVE for the shortest tail
                    copy_out(RoT[:, 640:1024], tp[:, 128:512], 0)
                    copy_out(RoT[:, 512:640], tp[:, 0:128], 0)

            # weight by e: bow[:, c*10:+10] = bb[:, ...] * e_c
            bow = sb.tile([128, 80], f32, tag="bow")
            for g in range(2):
                ev = e_sb[:, g * 4:(g + 1) * 4].rearrange(
                    "p (c o) -> p c o", o=1).to_broadcast((128, 4, 10))
                ov = bow[:, g * 40:(g + 1) * 40].rearrange(
                    "p (c j) -> p c j", j=10)
                iv = bb_ps[:, g * 40:(g + 1) * 40].rearrange(
                    "p (c j) -> p c j", j=10)
                nc.vector.tensor_tensor(ov, iv, ev, Alu.mult)

            # mi/mo accumulation: mm_ps cols 0-4 = mi, 5-9 = mo
            mi_ps = psac.tile([128, 5], f32, tag="mi")
            mo_ps = psac2.tile([128, 5], f32, tag="mo")
            for c in range(8):
                nc.tensor.matmul(mi_ps[:],
                                 RiT[:, c * 128:(c + 1) * 128],
                                 bow[:, c * 10:c * 10 + 5],
                                 start=(c == 0), stop=(c == 7))
                nc.tensor.matmul(mo_ps[:],
                                 RoT[:, c * 128:(c + 1) * 128],
                                 bow[:, c * 10 + 5:c * 10 + 10],
                                 start=(c == 0), stop=(c == 7))

            # ---------------- sincos B (critical path) ----------------
            # angle_q = mm_ps[:, q] + theta_q  (q = 0..7)
            rowB = sb.tile([128, 8], f32, tag="rowB")
            rowB2 = sb.tile([128, 8], f32, tag="rowB2")
            nc.vector.tensor_scalar(rowB[:], TH[:, 0:8], K4, 16.0,
                                    Alu.mult, Alu.add)
            nc.vector.tensor_scalar(rowB2[:], TH[:, 0:8], K4, 16.25,
                                    Alu.mult, Alu.add)

            wsB = sb.tile([128, 16], f32, tag="wsB")
            csB = sb.tile([128, 16], f32, tag="csB")
            nc.vector.scalar_tensor_tensor(wsB[:, 0:5], mi_ps[:], K4,
                                           rowB[:, 0:5], Alu.mult, Alu.add)
            nc.vector.scalar_tensor_tensor(wsB[:, 5:8], mo_ps[:, 0:3], K4,
                                           rowB[:, 5:8], Alu.mult, Alu.add)
            nc.vector.scalar_tensor_tensor(wsB[:, 8:13], mi_ps[:], K4,
                                           rowB2[:, 0:5], Alu.mult, Alu.add)
            nc.vector.scalar_tensor_tensor(wsB[:, 13:16], mo_ps[:, 0:3], K4,
                                           rowB2[:, 5:8], Alu.mult, Alu.add)
            kiB = sb.tile([128, 16], i32, tag="kiB")
            kfB = sb.tile([128, 16], f32, tag="kfB")
            nc.vector.tensor_copy(kiB[:], wsB[:])
            nc.vector.tensor_copy(kfB[:], kiB[:])
            nc.vector.tensor_tensor(wsB[:], wsB[:], kfB[:], Alu.subtract)
            nc.vector.tensor_scalar(kfB[:], wsB[:], 0.5, None, Alu.is_gt)
            nc.vector.scalar_tensor_tensor(wsB[:], kfB[:], -1.0, wsB[:],
                                           Alu.mult, Alu.add)
            nc.scalar.activation(csB[:], wsB[:], Act.Sin, scale=2.0 * PI)
            # csB: sin(q) at col q, cos(q) at col 8+q

            # ---------------- q5 evolution ----------------
            # Level 0 (batched): mA = {m0=(q0,q1), m2=(q4,q5)},
            #                    mB = {m1=(q3,q2), m3=(q7,q6)}
            # layout: col = comp + 2*b0 + 4*b1   (b0 = L bit, b1 = H bit)
            mA = sb.tile([128, 8], f32, tag="mA")
            mB = sb.tile([128, 8], f32, tag="mB")
            csBv = csB[:].rearrange("p (c d b) -> p c d b", d=2, b=4)

            def level0(mt, lq, hq, eng):
                # L pair cols (8+lq, lq); H pair (8+hq, hq); comps lq,lq+4
                tt = nc.vector.tensor_tensor if eng == 0 else \
                    nc.gpsimd.tensor_tensor
                ov = mt[:].rearrange("p (b1 b0 c) -> p b1 b0 c", b0=2, c=2)
                # operand views: col(H) = 8 - 8*b1 + 4*comp + hq
                Hb = csB[:].rearrange("p (c d b) -> p c d b", d=2, b=4)[
                    :, ::-1, :, hq:hq + 1].rearrange("p c d o -> p c (d o)")
                Hbr = csB[:].rearrange("p (c d b) -> p c d b", d=2, b=4)[
                    :, :, :, hq:hq + 1].rearrange("p c d o -> p c (d o)")
                Lcb = csBv[:, 1, :, lq:lq + 1].rearrange("p d o -> p (d o)")\
                    .unsqueeze(1).to_broadcast((128, 2, 2))
                Lsb = csBv[:, 0, :, lq:lq + 1].rearrange("p d o -> p (d o)")\
                    .unsqueeze(1).to_broadcast((128, 2, 2))
                tt(ov[:, :, 0, :], Lcb, Hb, Alu.mult)
                tt(ov[:, :, 1, :], Lsb, Hbr, Alu.mult)

            level0(mA, 0, 1, 0)   # m0=(q0 ctrl, q1 tgt), m2=(q4, q5) on DVE
            level0(mB, 3, 2, 1)   # m1=(q3, q2), m3=(q7, q6) on Pool

            # b1 RYs: mA comps (m0: th15, m2: th14) -> csA cols (1, 0)
            #         mB comps (m1: th16, m3: th15) -> csA cols (2, 1)
            sc4 = sb.tile([128, 4], f32, tag="sc4")
            sc4b = sb.tile([128, 4], f32, tag="sc4b")
            sc4c = sb.tile([128, 4], f32, tag="sc4c")
            sc4d = sb.tile([128, 4], f32, tag="sc4d")
            dA = sb.tile([128, 8], f32, tag="dA")
            dB = sb.tile([128, 8], f32, tag="dB")

            def ry_b1_batch(mt, c_hi, scv, scv2, dt, eng):
                # coefs: comp0 at csA col c_hi, comp1 at col c_hi-1
                tt = nc.vector.tensor_tensor if eng == 0 else \
                    nc.gpsimd.tensor_tensor
                cview = csA[:].rearrange("p (o x) -> p o x", o=1)[
                    :, :, 16 + c_hi - 1:16 + c_hi + 1][:, :, ::-1]
                sview = csA[:].rearrange("p (o x) -> p o x", o=1)[
                    :, :, c_hi - 1:c_hi + 1][:, :, ::-1]
                cb = cview.to_broadcast((128, 2, 2))
                sb_ = sview.unsqueeze(1).to_broadcast((128, 2, 2, 2))
                a0 = mt[:, 0:4].rearrange("p (b0 c) -> p b0 c", c=2)
                a1 = mt[:, 4:8].rearrange("p (b0 c) -> p b0 c", c=2)
                dv = dt[:].rearrange("p (b1 b0 c) -> p b1 b0 c", b0=2, c=2)
                t0 = scv[:].rearrange("p (b0 c) -> p b0 c", c=2)
                t1 = scv2[:].rearrange("p (b0 c) -> p b0 c", c=2)
                tt(t0, a0, cb, Alu.mult)
                tt(t1, a1, cb, Alu.mult)
                tt(dv, mt[:].rearrange("p (b1 b0 c) -> p b1 b0 c", b0=2, c=2),
                   sb_, Alu.mult)
                tt(a0, t0, dv[:, 1], Alu.subtract)
                tt(a1, t1, dv[:, 0], Alu.add)

            ry_b1_batch(mA, 1, sc4, sc4b, dA, 0)
            ry_b1_batch(mB, 2, sc4c, sc4d, dB, 1)

            # b0 RY on m0 (th25 = csA col 4): m0 = mA comp 0, strided
            m0v = mA[:].rearrange("p (b1 b0 c) -> p b1 b0 c", b0=2, c=2)
            nc.vector.tensor_scalar(
                dA[:].rearrange("p (b1 b0 c) -> p b1 b0 c", b0=2, c=2)
                [:, :, :, 0:1],
                m0v[:, :, :, 0:1], sA(4), None, Alu.mult)
            dAv = dA[:].rearrange("p (b1 b0 c) -> p b1 b0 c", b0=2, c=2)
            nc.vector.scalar_tensor_tensor(
                m0v[:, :, 0, 0:1], m0v[:, :, 0, 0:1], cA(4),
                dAv[:, :, 1, 0:1], Alu.mult, Alu.subtract)
            nc.vector.scalar_tensor_tensor(
                m0v[:, :, 1, 0:1], m0v[:, :, 1, 0:1], cA(4),
                dAv[:, :, 0, 0:1], Alu.mult, Alu.add)

            # m5 = merge+cnot(m0, m1; ctrl=b1 of m0, tgt=b1 of m1) on DVE
            # m6 = merge+cnot(m3, m2; same) on Pool
            # m56: m5 = cols 0-15, m6 = 16-31; col = l + 4*h
            m56 = sb.tile([128, 32], f32, tag="m56")

            def merge_cnot_l1h3(dst_off, Lt, l_comp, Ht, h_comp, eng):
                tt = nc.vector.tensor_tensor if eng == 0 else \
                    nc.gpsimd.tensor_tensor
                # L[cb, v0] at col l_comp + 2*v0 + 4*cb
                Lv = Lt[:].rearrange("p (cb v0 c) -> p cb v0 c", v0=2, c=2)
                Hv = Ht[:].rearrange("p (hb1 hb0 c) -> p hb1 hb0 c",
                                     hb0=2, c=2)
                ov = m56[:, dst_off:dst_off + 16].rearrange(
                    "p (hb1 hb0 cb v0) -> p hb1 hb0 cb v0", hb0=2, cb=2, v0=2)
                # cb=0: out = L[0, v0] * H[hb1, hb0]
                tt(ov[:, :, :, 0, :],
                   Lv[:, 0, :, l_comp:l_comp + 1].rearrange("p v o -> p (v o)")
                   .unsqueeze(1).unsqueeze(1).to_broadcast((128, 2, 2, 2)),
                   Hv[:, :, :, h_comp:h_comp + 1]
                   .to_broadcast((128, 2, 2, 2)),
                   Alu.mult)
                # cb=1: out = L[1, v0] * H[1-hb1, hb0]
                tt(ov[:, :, :, 1, :],
                   Lv[:, 1, :, l_comp:l_comp + 1].rearrange("p v o -> p (v o)")
                   .unsqueeze(1).unsqueeze(1).to_broadcast((128, 2, 2, 2)),
                   Hv[:, ::-1, :, h_comp:h_comp + 1]
                   .to_broadcast((128, 2, 2, 2)),
                   Alu.mult)

            merge_cnot_l1h3(0, mA, 0, mB, 0, 0)    # m5 on DVE
            merge_cnot_l1h3(16, mB, 1, mA, 1, 1)   # m6 on Pool

            # RY(m5, b3, th19 = csA col 3) on DVE
            d16 = sb.tile([128, 16], f32, tag="d16")
            nc.vector.tensor_scalar(d16[:], m56[:, 0:16], sA(3), None,
                                    Alu.mult)
            nc.vector.scalar_tensor_tensor(
                m56[:, 0:8], m56[:, 0:8], cA(3), d16[:, 8:16],
                Alu.mult, Alu.subtract)
            nc.vector.scalar_tensor_tensor(
                m56[:, 8:16], m56[:, 8:16], cA(3), d16[:, 0:8],
                Alu.mult, Alu.add)

            # H0 = RY(th20+a)(m6), G = RY(th20-a)(m6)  [a = th23+th26]
            # csA col 7 = (th20+a), col 8 = (th20-a); on Pool
            h0t = sb.tile([128, 16], f32, tag="h0t")
            gt = sb.tile([128, 16], f32, tag="gt")
            da = sb.tile([128, 16], f32, tag="da")
            db = sb.tile([128, 16], f32, tag="db")
            da2 = sb.tile([128, 16], f32, tag="da2")
            db2 = sb.tile([128, 16], f32, tag="db2")
            m6v = m56[:, 16:32]
            nc.gpsimd.tensor_tensor(da[:], m6v,
                                    sA(7).to_broadcast((128, 16)), Alu.mult)
            nc.gpsimd.tensor_tensor(db[:], m6v,
                                    cA(7).to_broadcast((128, 16)), Alu.mult)
            nc.gpsimd.tensor_tensor(h0t[:, 0:8], db[:, 0:8], da[:, 8:16],
                                    Alu.subtract)
            nc.gpsimd.tensor_tensor(h0t[:, 8:16], da[:, 0:8], db[:, 8:16],
                                    Alu.add)
            nc.gpsimd.tensor_tensor(da2[:], m6v,
                                    sA(8).to_broadcast((128, 16)), Alu.mult)
            nc.gpsimd.tensor_tensor(db2[:], m6v,
                                    cA(8).to_broadcast((128, 16)), Alu.mult)
            nc.gpsimd.tensor_tensor(gt[:, 0:8], db2[:, 0:8], da2[:, 8:16],
                                    Alu.subtract)
            nc.gpsimd.tensor_tensor(gt[:, 8:16], da2[:, 0:8], db2[:, 8:16],
                                    Alu.add)

            # m7 = merge: cols l + 16*h; cb = m5 b3 (l in 8-15)
            m7 = sb.tile([128, 256], f32, tag="m7")
            m7v = m7[:].rearrange("p (h l) -> p h l", l=16)
            nc.vector.tensor_tensor(
                m7v[:, :, 0:8],
                m56[:, 0:8].unsqueeze(1).to_broadcast((128, 16, 8)),
                h0t[:].unsqueeze(2).to_broadcast((128, 16, 8)),
                Alu.mult)
            m7v2 = m7[:].rearrange("p (hb hl l) -> p hb hl l", hl=8, l=16)
            nc.gpsimd.tensor_tensor(
                m7v2[:, :, :, 8:16],
                m56[:, 8:16].unsqueeze(1).unsqueeze(1)
                .to_broadcast((128, 2, 8, 8)),
                gt[:].rearrange("p (b x) -> p b x", b=2)[:, ::-1, :]
                .unsqueeze(3).to_broadcast((128, 2, 8, 8)),
                Alu.mult)

            # measurement: z = sum a0*(Rz a0 + s29 a1) + sum a1*(Rzn a1
            # + s29 a0);  a0 = m7[:, 0:128], a1 = m7[:, 128:256]
            w0 = sb.tile([128, 128], f32, tag="w0")
            w1 = sb.tile([128, 128], f32, tag="w1")
            w1b = sb.tile([128, 128], f32, tag="w1b")
            a0 = m7[:, 0:128]
            a1 = m7[:, 128:256]
            nc.vector.tensor_tensor(w0[:], a0, Rz[:], Alu.mult)
            nc.vector.scalar_tensor_tensor(w0[:], a1, nsA[:, 0:1], w0[:],
                                           Alu.mult, Alu.add)
            nc.vector.scalar_tensor_tensor(w0[:], a0, 1.0, w0[:],
                                           Alu.mult, Alu.mult,
                                           accum_out=zac[:, 5:6])
            import concourse.mybir as _mb
            nc.gpsimd.tensor_tensor(w1[:], a1, Rzn[:], Alu.mult)
            nc.gpsimd.tensor_tensor(w1b[:], a0,
                                    nsA[:, 0:1].to_broadcast((128, 128)),
                                    Alu.mult)
            nc.gpsimd.tensor_tensor(w1[:], w1[:], w1b[:], Alu.add)
            nc.gpsimd.tensor_tensor(w1[:], a1, w1[:], Alu.mult)
            nc.vector.tensor_reduce(zac[:, 6:7], w1[:],
                                    _mb.AxisListType.X, Alu.add)
            nc.vector.tensor_tensor(zac[:, 7:8], zac[:, 5:6], zac[:, 6:7],
                                    Alu.add)
            nc.vector.tensor_scalar(out_sb[:, 0:1], zac[:, 7:8], -PI, PI,
                                    Alu.mult, Alu.add)

            nc.sync.dma_start(out_d, out_sb[:])

    nc.compile()
    return nc


def get_nc():
    if "nc" not in _cache:
        _cache["nc"] = _build_program()
    return _cache["nc"]


def kernel(X, e, Ri, Ro, theta):
    from concourse.bass_utils import run_bass_kernel_spmd

    nc = get_nc()
    in_map = {
        "X": np.ascontiguousarray(np.asarray(X, dtype=np.float32)),
        "e": np.ascontiguousarray(np.asarray(e, dtype=np.float32)),
        "Ri": np.ascontiguousarray(np.asarray(Ri, dtype=np.float32)),
        "Ro": np.ascontiguousarray(np.asarray(Ro, dtype=np.float32)),
        "theta": np.ascontiguousarray(np.asarray(theta, dtype=np.float32)),
    }
    res = run_bass_kernel_spmd(
        nc, [dict(in_map) for _ in range(N_CORES)],
        core_ids=list(range(N_CORES)),
    )
    return res.results[0]["out"]


# revision 25
# speedup vs baseline: 1.0481x; 1.0481x over previous
"""Trainium2 Bass kernel for nn_NodeNet (GNN message passing + 15-qubit circuit).

Exact algebraic structure exploited (hand-scheduled version):
1. The 2^15 state stays a tensor product of small components; only the
   q5 component reaches 256 dims and only it depends on the message-passing
   matmuls.  The q10 measurement chain depends on X only, so it runs
   entirely inside the input-DMA window.
2. Final CNOT+RY before each measurement are folded into the observable
   (Heisenberg picture): O = cos(th)*Z_c Z_t + sin(th)*X_t, measured with
   3 fused multiply-accumulate ops instead of gate applications.
3. The last CNOT(3,7)+RY(b7) pair on the q5 chain folds into the m5 x m6
   merge by pre-rotating m6 two ways (theta20+-alpha) - the 256-wide RY
   disappears.
4. Range reduction for sin/cos uses mod: sin(u/2) = Sin(pi - 2pi*frac(
   u/(4pi)+16)), one fused ACT op produces both sin and cos columns.
5. DMA plan minimizes HWDGE serialization (a single device in HW): big
   matrices split between the SP HWDGE queue and the Pool SWDGE queue,
   the last-arriving piece is only 128 columns wide.

Self-contained: hardcodes shapes (N=128, E=1024) and the (pre-simplified)
gate structure.
"""

import math

import numpy as np

N_CORES = 8
PI = math.pi

_cache = {}


def _build_program():
    import concourse.bacc as bacc
    import concourse.mybir as mybir
    import concourse.tile as tile
    from concourse.masks import make_identity

    f32 = mybir.dt.float32
    i32 = mybir.dt.int32
    Alu = mybir.AluOpType
    Act = mybir.ActivationFunctionType

    nc = bacc.Bacc(
        "TRN2",
        target_bir_lowering=False,
        debug=False,
        enable_asserts=False,
        num_devices=1,
    )

    X_d = nc.dram_tensor("X", [128, 5], f32, kind="ExternalInput").ap()
    e_d = nc.dram_tensor("e", [1024], f32, kind="ExternalInput").ap()
    Ri_d = nc.dram_tensor("Ri", [128, 1024], f32, kind="ExternalInput").ap()
    Ro_d = nc.dram_tensor("Ro", [128, 1024], f32, kind="ExternalInput").ap()
    th_d = nc.dram_tensor("theta", [31], f32, kind="ExternalInput").ap()
    out_d = nc.dram_tensor("out", [128, 2], f32, kind="ExternalOutput").ap()

    with tile.TileContext(nc) as tc:
        with (
            tc.tile_pool(name="sbuf", bufs=1) as sb,
            tc.tile_pool(name="psmm", bufs=1, space="PSUM") as psmm,
            tc.tile_pool(name="pstp", bufs=1, space="PSUM") as pstp,
            tc.tile_pool(name="psac", bufs=1, space="PSUM") as psac,
            tc.tile_pool(name="psac2", bufs=1, space="PSUM") as psac2,
        ):
            # ---------------- SBUF tiles ----------------
            Ri_sb = sb.tile([128, 1024], f32, tag="Ri")
            Ro_sb = sb.tile([128, 1024], f32, tag="Ro")
            RiT = sb.tile([128, 1024], f32, tag="RiT")
            RoT = sb.tile([128, 1024], f32, tag="RoT")
            X_sb = sb.tile([128, 5], f32, tag="X")
            TH = sb.tile([128, 31], f32, tag="TH")
            e8_sb = sb.tile([8, 128], f32, tag="e8")
            e_sb = sb.tile([128, 8], f32, tag="e_sb")
            ident = sb.tile([128, 128], f32, tag="ident")

            # ---------------- DMA dispatches ----------------
            # SP HWDGE queue (in order): Ri halves first (earliest big
            # transfers), then theta (broadcast to 128 partitions), e8,
            # and the tail 384 cols of Ro.
            th1 = sb.tile([1, 31], f32, tag="th1")
            nc.sync.dma_start(th1[:], th_d.unsqueeze(0))
            nc.sync.dma_start(Ri_sb[:, 0:512], Ri_d[:, 0:512])
            nc.sync.dma_start(Ri_sb[:, 512:1024], Ri_d[:, 512:1024])
            nc.sync.dma_start(e8_sb[:], e_d.rearrange("(c p) -> c p", c=8))
            nc.sync.dma_start(Ro_sb[:, 640:1024], Ro_d[:, 640:1024])

            # Pool SWDGE queue: X, then Ro cols 0-511, then the small
            # 128-col piece (arrives last; short matmul tail).
            nc.gpsimd.dma_start(X_sb[:], X_d)
            make_identity(nc, ident[:])
            nc.gpsimd.dma_start(Ro_sb[:, 0:512], Ro_d[:, 0:512])
            nc.gpsimd.dma_start(Ro_sb[:, 512:640], Ro_d[:, 512:640])

            # theta broadcast to all partitions via K=1 matmul
            ones1 = sb.tile([1, 128], f32, tag="ones1")
            nc.vector.memset(ones1[:], 1.0)
            th_ps = psac.tile([128, 31], f32, tag="th_ps")
            nc.tensor.matmul(th_ps[:], ones1[:], th1[:], start=True, stop=True)
            nc.vector.tensor_copy(TH[:], th_ps[:])

            # ---------------- constants / warmup ----------------
            # Preload the Sin activation table during the DMA window.
            warm = sb.tile([1, 1], f32, tag="warm")
            nc.vector.memset(warm[:], 0.0)
            nc.scalar.activation(warm[:], warm[:], Act.Sin)

            # sign row sigma[x] = (-1)^x, replicated across partitions
            sigI = sb.tile([128, 128], i32, tag="sigI")
            sigF = sb.tile([128, 128], f32, tag="sigF")
            nc.gpsimd.iota(sigI[:], pattern=[[1, 128]], base=0,
                           channel_multiplier=0)
            nc.vector.tensor_scalar(sigI[:], sigI[:], 1, None, Alu.bitwise_and)
            nc.vector.tensor_copy(sigF[:], sigI[:])
            nc.vector.tensor_scalar(sigF[:], sigF[:], -2.0, 1.0,
                                    Alu.mult, Alu.add)

            # ---------------- angle block A (theta + X qubits) ----------
            # AANG columns (full angles u; we produce cos/sin of u/2):
            #  0: th14   1: th15   2: th16   3: th19   4: th25
            #  5: th17+th21        6: th24+th27
            #  7: th20+th23+th26   8: th20-th23-th26
            #  9: 2*th29          10: 2*th30
            # 11: X0+th10  12: X1+th11  13: X3+th13+th18+th22
            # 14: X4+th14+th19+th28    15: 2*col14
            _hp = tc.high_priority()
            _hp.__enter__()
            AANG = sb.tile([128, 16], f32, tag="AANG")
            scr2 = sb.tile([128, 2], f32, tag="scr2")

            THv = TH[:].rearrange("p (o x) -> p o x", o=1)

            def thcols(lo, n, step=1):
                return THv[:, :, lo:lo + (n - 1) * step + 1:step] if step > 1 \
                    else TH[:, lo:lo + n]

            AAv = AANG[:].rearrange("p (o x) -> p o x", o=1)

            nc.vector.tensor_copy(AANG[:, 0:3], TH[:, 14:17])
            nc.vector.tensor_copy(AAv[:, :, 3:5], thcols(19, 2, 6))
            nc.vector.tensor_tensor(AAv[:, :, 5:7], thcols(17, 2, 7),
                                    thcols(21, 2, 6), Alu.add)
            nc.vector.tensor_tensor(scr2[:, 0:1], TH[:, 23:24], TH[:, 26:27],
                                    Alu.add)
            nc.vector.tensor_tensor(AANG[:, 7:8], TH[:, 20:21], scr2[:, 0:1],
                                    Alu.add)
            nc.vector.tensor_tensor(AANG[:, 8:9], TH[:, 20:21], scr2[:, 0:1],
                                    Alu.subtract)
            nc.vector.tensor_tensor(AANG[:, 9:11], TH[:, 29:31], TH[:, 29:31],
                                    Alu.add)
            nc.vector.tensor_tensor(AANG[:, 11:13], X_sb[:, 0:2], TH[:, 10:12],
                                    Alu.add)
            nc.vector.tensor_tensor(scr2[:], TH[:, 13:15], TH[:, 18:20],
                                    Alu.add)
            nc.vector.tensor_tensor(scr2[:].rearrange("p (o x) -> p o x", o=1),
                                    scr2[:].rearrange("p (o x) -> p o x", o=1),
                                    thcols(22, 2, 6), Alu.add)
            nc.vector.tensor_tensor(AANG[:, 13:15], X_sb[:, 3:5], scr2[:],
                                    Alu.add)
            nc.vector.tensor_tensor(AANG[:, 15:16], AANG[:, 14:15],
                                    AANG[:, 14:15], Alu.add)

            # sincos A: csA[:, j] = sin(u_j/2), csA[:, 16+j] = cos(u_j/2)
            wsA = sb.tile([128, 32], f32, tag="wsA")
            csA = sb.tile([128, 32], f32, tag="csA")
            K4 = 1.0 / (4.0 * PI)
            nc.vector.tensor_scalar(wsA[:, 0:16], AANG[:], K4, 16.0,
                                    Alu.mult, Alu.add)
            nc.vector.tensor_scalar(wsA[:, 16:32], AANG[:], K4, 16.25,
                                    Alu.mult, Alu.add)
            kiA = sb.tile([128, 32], i32, tag="kiA")
            kfA = sb.tile([128, 32], f32, tag="kfA")
            nc.vector.tensor_copy(kiA[:], wsA[:])
            nc.vector.tensor_copy(kfA[:], kiA[:])
            nc.vector.tensor_tensor(wsA[:], wsA[:], kfA[:], Alu.subtract)
            nc.vector.tensor_scalar(kfA[:], wsA[:], 0.5, None, Alu.is_gt)
            nc.vector.scalar_tensor_tensor(wsA[:], kfA[:], -1.0, wsA[:],
                                           Alu.mult, Alu.add)
            nc.scalar.activation(csA[:], wsA[:], Act.Sin, scale=2.0 * PI)

            def sA(j):
                return csA[:, j:j + 1]

            def cA(j):
                return csA[:, 16 + j:16 + j + 1]

            # negated sins for the folded measurements
            nsA = sb.tile([128, 2], f32, tag="nsA")
            nc.vector.tensor_scalar(nsA[:], csA[:, 9:11], -1.0, None, Alu.mult)
            _hp.__exit__(None, None, None)

            # measurement rows (ready early): Rz = cos(th29)*sigma,
            # Rzneg = -Rz
            Rz = sb.tile([128, 128], f32, tag="Rz")
            Rzn = sb.tile([128, 128], f32, tag="Rzn")
            nc.vector.tensor_scalar(Rz[:], sigF[:], cA(9), None, Alu.mult)
            nc.vector.tensor_scalar(Rzn[:], Rz[:], -1.0, None, Alu.mult)

            # ---------------- q10 chain (X-only, hidden under DMA) -------
            # m4 = merge+cnot(q11(L), q10(H)); RY(b1, col5); m8 = merge+
            # cnot(q13(L), m4(H)); RY(b2, col6); measure folded:
            # z10 = c30*cos(q14_full)*<Z_b2> + 2*s30*<X_b2-pair>
            m4 = sb.tile([128, 4], f32, tag="m4")
            m8 = sb.tile([128, 8], f32, tag="m8")
            d4 = sb.tile([128, 4], f32, tag="d4")
            d8 = sb.tile([128, 8], f32, tag="d8")
            zac = sb.tile([128, 8], f32, tag="zac")
            out_sb = sb.tile([128, 2], f32, tag="out")

            csAv = csA[:].rearrange("p (c d b) -> p c d b", d=2, b=8)
            # L=q11 pair: cols (16+12, 12); H=q10 pair: (16+11, 11)
            m4v = m4[:].rearrange("p (t c) -> p t c", c=2)
            # H[t]: t=0 -> col 27, t=1 -> col 11  (start 27 stride -16)
            Hq10 = csA[:].rearrange("p (c x) -> p c x", c=2)[:, ::-1, 11:12]
            Hq10r = csA[:].rearrange("p (c x) -> p c x", c=2)[:, :, 11:12]
            nc.vector.tensor_tensor(
                m4v[:, :, 0:1], Hq10,
                cA(12).unsqueeze(1).to_broadcast((128, 2, 1)), Alu.mult)
            nc.vector.tensor_tensor(
                m4v[:, :, 1:2], Hq10r,
                sA(12).unsqueeze(1).to_broadcast((128, 2, 1)), Alu.mult)
            # RY(m4, b1, col5): a0=m4[:,0:2] (t=0), a1=m4[:,2:4]
            nc.vector.tensor_scalar(d4[:], m4[:], sA(5), None, Alu.mult)
            nc.vector.scalar_tensor_tensor(
                m4[:, 0:2], m4[:, 0:2], cA(5), d4[:, 2:4],
                Alu.mult, Alu.subtract)
            nc.vector.scalar_tensor_tensor(
                m4[:, 2:4], m4[:, 2:4], cA(5), d4[:, 0:2],
                Alu.mult, Alu.add)
            # m8 = merge+cnot(q13(L,b0), m4(H,b1-b2)); ctrl b0, tgt b2
            m8v = m8[:].rearrange("p (h c) -> p h c", c=2)
            m4f = m4[:].rearrange("p (t i) -> p t i", t=2)
            nc.vector.tensor_tensor(
                m8v[:, :, 0:1], m4[:].unsqueeze(2),
                cA(13).unsqueeze(1).to_broadcast((128, 4, 1)), Alu.mult)
            m8v2 = m8[:].rearrange("p (hb1 hb0 c) -> p hb1 hb0 c",
                                   hb0=2, c=2)
            nc.vector.tensor_tensor(
                m8v2[:, :, :, 1:2],
                m4f[:, ::-1, :].unsqueeze(3),
                sA(13).unsqueeze(1).unsqueeze(1)
                .to_broadcast((128, 2, 2, 1)), Alu.mult)
            # RY(m8, b2, col6): a0 = cols 0-3, a1 = cols 4-7
            nc.vector.tensor_scalar(d8[:], m8[:], sA(6), None, Alu.mult)
            nc.vector.scalar_tensor_tensor(
                m8[:, 0:4], m8[:, 0:4], cA(6), d8[:, 4:8],
                Alu.mult, Alu.subtract)
            nc.vector.scalar_tensor_tensor(
                m8[:, 4:8], m8[:, 4:8], cA(6), d8[:, 0:4],
                Alu.mult, Alu.add)
            # measure: w0=sum(a0^2), w1=sum(a1^2), w2=sum(a0*a1)
            nc.vector.scalar_tensor_tensor(d8[:, 0:4], m8[:, 0:4], 1.0,
                                           m8[:, 0:4], Alu.mult, Alu.mult,
                                           accum_out=zac[:, 0:1])
            nc.vector.scalar_tensor_tensor(d8[:, 4:8], m8[:, 4:8], 1.0,
                                           m8[:, 4:8], Alu.mult, Alu.mult,
                                           accum_out=zac[:, 1:2])
            nc.vector.scalar_tensor_tensor(d4[:, 0:4], m8[:, 0:4], 1.0,
                                           m8[:, 4:8], Alu.mult, Alu.mult,
                                           accum_out=zac[:, 2:3])
            # z10 = cA(10)*cA(15)*(w0-w1) - 2*sA(10)*w2
            nc.vector.tensor_tensor(zac[:, 3:4], zac[:, 0:1], zac[:, 1:2],
                                    Alu.subtract)
            nc.vector.tensor_scalar(zac[:, 3:4], zac[:, 3:4], cA(15), cA(10),
                                    Alu.mult, Alu.mult)
            nc.vector.tensor_tensor(zac[:, 4:5], zac[:, 2:3], zac[:, 2:3],
                                    Alu.add)
            nc.vector.scalar_tensor_tensor(
                zac[:, 4:5], zac[:, 4:5], nsA[:, 1:2], zac[:, 3:4],
                Alu.mult, Alu.add)
            nc.vector.tensor_scalar(out_sb[:, 1:2], zac[:, 4:5], -PI, PI,
                                    Alu.mult, Alu.add)

            # ---------------- message passing pipeline ----------------
            # e transpose: [8,128] -> [128,8]
            e_ps = psmm.tile([128, 8], f32, tag="e_ps")
            nc.tensor.transpose(e_ps[:], e8_sb[:], ident[0:8, 0:8])
            nc.scalar.copy(e_sb[:], e_ps[:])

            # bo/bi: bb_ps[:, c*10:+5] = Ro_c^T X ; +5:+10 = Ri_c^T X
            bb_ps = psmm.tile([128, 80], f32, tag="bb")
            for c in range(8):
                nc.tensor.matmul(bb_ps[:, c * 10 + 5:c * 10 + 10],
                                 Ri_sb[:, c * 128:(c + 1) * 128], X_sb[:],
                                 start=True, stop=True)
            for c in range(8):
                nc.tensor.matmul(bb_ps[:, c * 10:c * 10 + 5],
                                 Ro_sb[:, c * 128:(c + 1) * 128], X_sb[:],
                                 start=True, stop=True)

            # transposes: Ri chunks then Ro chunks; copies alternate
            # DVE / ACT / Pool.
            def copy_out(dst, src, eng):
                if eng == 0:
                    nc.vector.tensor_copy(dst, src)
                elif eng == 1:
                    nc.scalar.copy(dst, src)
                else:
                    nc.gpsimd.tensor_copy(dst, src)

            tpri_tiles = {}
            for h in range(2):
                tp = pstp.tile([128, 512], f32, tag=f"tpri{h}")
                tpri_tiles[h] = tp
                for cc in range(4):
                    c = h * 4 + cc
                    nc.tensor.transpose(tp[:, cc * 128:(cc + 1) * 128],
                                        Ri_sb[:, c * 128:(c + 1) * 128],
                                        ident[:])
                copy_out(RiT[:, h * 512:h * 512 + 256], tp[:, 0:256], 1)
                copy_out(RiT[:, h * 512 + 256:h * 512 + 512], tp[:, 256:512],
                         1)
            for h in range(2):
                tp = tpri_tiles[0] if h == 1 else \
                    pstp.tile([128, 512], f32, tag="tpro0")
                for cc in range(4):
                    c = h * 4 + cc
                    nc.tensor.transpose(tp[:, cc * 128:(cc + 1) * 128],
                                        Ro_sb[:, c * 128:(c + 1) * 128],
                                        ident[:])
                if h == 0:
                    copy_out(RoT[:, 0:256], tp[:, 0:256], 1)
                    copy_out(RoT[:, 256:512], tp[:, 256:512], 1)
                else:
                    # last piece 512:640 arrives latest - copy its
                    # transpose separately on DVE for the shortest tail
                    copy_out(RoT[:, 640:1024], tp[:, 128:512], 1)
                    copy_out(RoT[:, 512:640], tp[:, 0:128], 0)

            # weight by e: bow[:, c*10:+10] = bb[:, ...] * e_c
            bow = sb.tile([128, 80], f32, tag="bow")
            for g in range(2):
                ev = e_sb[:, g * 4:(g + 1) * 4].rearrange(
                    "p (c o) -> p c o", o=1).to_broadcast((128, 4, 10))
                ov = bow[:, g * 40:(g + 1) * 40].rearrange(
                    "p (c j) -> p c j", j=10)
                iv = bb_ps[:, g * 40:(g + 1) * 40].rearrange(
                    "p (c j) -> p c j", j=10)
                nc.vector.tensor_tensor(ov, iv, ev, Alu.mult)

            # mi/mo accumulation: mm_ps cols 0-4 = mi, 5-9 = mo
            mi_ps = psac.tile([128, 5], f32, tag="mi")
            mo_ps = psac2.tile([128, 5], f32, tag="mo")
            for c in range(8):
                nc.tensor.matmul(mi_ps[:],
                                 RiT[:, c * 128:(c + 1) * 128],
                                 bow[:, c * 10:c * 10 + 5],
                                 start=(c == 0), stop=(c == 7))
                nc.tensor.matmul(mo_ps[:],
                                 RoT[:, c * 128:(c + 1) * 128],
                                 bow[:, c * 10 + 5:c * 10 + 10],
                                 start=(c == 0), stop=(c == 7))

            # ---------------- sincos B (critical path) ----------------
            # angle_q = mm_ps[:, q] + theta_q  (q = 0..7)
            rowB = sb.tile([128, 8], f32, tag="rowB")
            rowB2 = sb.tile([128, 8], f32, tag="rowB2")
            nc.vector.tensor_scalar(rowB[:], TH[:, 0:8], K4, 16.0,
                                    Alu.mult, Alu.add)
            nc.vector.tensor_scalar(rowB2[:], TH[:, 0:8], K4, 16.25,
                                    Alu.mult, Alu.add)

            wsB = sb.tile([128, 16], f32, tag="wsB")
            csB = sb.tile([128, 16], f32, tag="csB")
            nc.vector.scalar_tensor_tensor(wsB[:, 0:5], mi_ps[:], K4,
                                           rowB[:, 0:5], Alu.mult, Alu.add)
            nc.vector.scalar_tensor_tensor(wsB[:, 5:8], mo_ps[:, 0:3], K4,
                                           rowB[:, 5:8], Alu.mult, Alu.add)
            nc.vector.scalar_tensor_tensor(wsB[:, 8:13], mi_ps[:], K4,
                                           rowB2[:, 0:5], Alu.mult, Alu.add)
            nc.vector.scalar_tensor_tensor(wsB[:, 13:16], mo_ps[:, 0:3], K4,
                                           rowB2[:, 5:8], Alu.mult, Alu.add)
            kiB = sb.tile([128, 16], i32, tag="kiB")
            kfB = sb.tile([128, 16], f32, tag="kfB")
            nc.vector.tensor_copy(kiB[:], wsB[:])
            nc.vector.tensor_copy(kfB[:], kiB[:])
            nc.vector.tensor_tensor(wsB[:], wsB[:], kfB[:], Alu.subtract)
            nc.vector.tensor_scalar(kfB[:], wsB[:], 0.5, None, Alu.is_gt)
            nc.vector.scalar_tensor_tensor(wsB[:], kfB[:], -1.0, wsB[:],
                                           Alu.mult, Alu.add)
            nc.scalar.activation(csB[:], wsB[:], Act.Sin, scale=2.0 * PI)
            # csB: sin(q) at col q, cos(q) at col 8+q

            # ---------------- q5 evolution ----------------
            # Level 0 (batched): mA = {m0=(q0,q1), m2=(q4,q5)},
            #                    mB = {m1=(q3,q2), m3=(q7,q6)}
            # layout: col = comp + 2*b0 + 4*b1   (b0 = L bit, b1 = H bit)
            mA = sb.tile([128, 8], f32, tag="mA")
            mB = sb.tile([128, 8], f32, tag="mB")
            csBv = csB[:].rearrange("p (c d b) -> p c d b", d=2, b=4)

            def level0(mt, lq, hq, eng):
                # L pair cols (8+lq, lq); H pair (8+hq, hq); comps lq,lq+4
                tt = nc.vector.tensor_tensor if eng == 0 else \
                    nc.gpsimd.tensor_tensor
                ov = mt[:].rearrange("p (b1 b0 c) -> p b1 b0 c", b0=2, c=2)
                # operand views: col(H) = 8 - 8*b1 + 4*comp + hq
                Hb = csB[:].rearrange("p (c d b) -> p c d b", d=2, b=4)[
                    :, ::-1, :, hq:hq + 1].rearrange("p c d o -> p c (d o)")
                Hbr = csB[:].rearrange("p (c d b) -> p c d b", d=2, b=4)[
                    :, :, :, hq:hq + 1].rearrange("p c d o -> p c (d o)")
                Lcb = csBv[:, 1, :, lq:lq + 1].rearrange("p d o -> p (d o)")\
                    .unsqueeze(1).to_broadcast((128, 2, 2))
                Lsb = csBv[:, 0, :, lq:lq + 1].rearrange("p d o -> p (d o)")\
                    .unsqueeze(1).to_broadcast((128, 2, 2))
                tt(ov[:, :, 0, :], Lcb, Hb, Alu.mult)
                tt(ov[:, :, 1, :], Lsb, Hbr, Alu.mult)

            level0(mA, 0, 1, 0)   # m0=(q0 ctrl, q1 tgt), m2=(q4, q5) on DVE
            level0(mB, 3, 2, 1)   # m1=(q3, q2), m3=(q7, q6) on Pool

            # b1 RYs: mA comps (m0: th15, m2: th14) -> csA cols (1, 0)
            #         mB comps (m1: th16, m3: th15) -> csA cols (2, 1)
            sc4 = sb.tile([128, 4], f32, tag="sc4")
            sc4b = sb.tile([128, 4], f32, tag="sc4b")
            sc4c = sb.tile([128, 4], f32, tag="sc4c")
            sc4d = sb.tile([128, 4], f32, tag="sc4d")
            dA = sb.tile([128, 8], f32, tag="dA")
            dB = sb.tile([128, 8], f32, tag="dB")

            def ry_b1_batch(mt, c_hi, scv, scv2, dt, eng):
                # coefs: comp0 at csA col c_hi, comp1 at col c_hi-1
                tt = nc.vector.tensor_tensor if eng == 0 else \
                    nc.gpsimd.tensor_tensor
                cview = csA[:].rearrange("p (o x) -> p o x", o=1)[
                    :, :, 16 + c_hi - 1:16 + c_hi + 1][:, :, ::-1]
                sview = csA[:].rearrange("p (o x) -> p o x", o=1)[
                    :, :, c_hi - 1:c_hi + 1][:, :, ::-1]
                cb = cview.to_broadcast((128, 2, 2))
                sb_ = sview.unsqueeze(1).to_broadcast((128, 2, 2, 2))
                a0 = mt[:, 0:4].rearrange("p (b0 c) -> p b0 c", c=2)
                a1 = mt[:, 4:8].rearrange("p (b0 c) -> p b0 c", c=2)
                dv = dt[:].rearrange("p (b1 b0 c) -> p b1 b0 c", b0=2, c=2)
                t0 = scv[:].rearrange("p (b0 c) -> p b0 c", c=2)
                t1 = scv2[:].rearrange("p (b0 c) -> p b0 c", c=2)
                tt(t0, a0, cb, Alu.mult)
                tt(t1, a1, cb, Alu.mult)
                tt(dv, mt[:].rearrange("p (b1 b0 c) -> p b1 b0 c", b0=2, c=2),
                   sb_, Alu.mult)
                tt(a0, t0, dv[:, 1], Alu.subtract)
                tt(a1, t1, dv[:, 0], Alu.add)

            ry_b1_batch(mA, 1, sc4, sc4b, dA, 0)
            ry_b1_batch(mB, 2, sc4c, sc4d, dB, 1)

            # b0 RY on m0 (th25 = csA col 4): m0 = mA comp 0, strided
            m0v = mA[:].rearrange("p (b1 b0 c) -> p b1 b0 c", b0=2, c=2)
            nc.vector.tensor_scalar(
                dA[:].rearrange("p (b1 b0 c) -> p b1 b0 c", b0=2, c=2)
                [:, :, :, 0:1],
                m0v[:, :, :, 0:1], sA(4), None, Alu.mult)
            dAv = dA[:].rearrange("p (b1 b0 c) -> p b1 b0 c", b0=2, c=2)
            nc.vector.scalar_tensor_tensor(
                m0v[:, :, 0, 0:1], m0v[:, :, 0, 0:1], cA(4),
                dAv[:, :, 1, 0:1], Alu.mult, Alu.subtract)
            nc.vector.scalar_tensor_tensor(
                m0v[:, :, 1, 0:1], m0v[:, :, 1, 0:1], cA(4),
                dAv[:, :, 0, 0:1], Alu.mult, Alu.add)

            # m5 = merge+cnot(m0, m1; ctrl=b1 of m0, tgt=b1 of m1) on DVE
            # m6 = merge+cnot(m3, m2; same) on Pool
            # m56: m5 = cols 0-15, m6 = 16-31; col = l + 4*h
            m56 = sb.tile([128, 32], f32, tag="m56")

            def merge_cnot_l1h3(dst_off, Lt, l_comp, Ht, h_comp, eng):
                tt = nc.vector.tensor_tensor if eng == 0 else \
                    nc.gpsimd.tensor_tensor
                # L[cb, v0] at col l_comp + 2*v0 + 4*cb
                Lv = Lt[:].rearrange("p (cb v0 c) -> p cb v0 c", v0=2, c=2)
                Hv = Ht[:].rearrange("p (hb1 hb0 c) -> p hb1 hb0 c",
                                     hb0=2, c=2)
                ov = m56[:, dst_off:dst_off + 16].rearrange(
                    "p (hb1 hb0 cb v0) -> p hb1 hb0 cb v0", hb0=2, cb=2, v0=2)
                # cb=0: out = L[0, v0] * H[hb1, hb0]
                tt(ov[:, :, :, 0, :],
                   Lv[:, 0, :, l_comp:l_comp + 1].rearrange("p v o -> p (v o)")
                   .unsqueeze(1).unsqueeze(1).to_broadcast((128, 2, 2, 2)),
                   Hv[:, :, :, h_comp:h_comp + 1]
                   .to_broadcast((128, 2, 2, 2)),
                   Alu.mult)
                # cb=1: out = L[1, v0] * H[1-hb1, hb0]
                tt(ov[:, :, :, 1, :],
                   Lv[:, 1, :, l_comp:l_comp + 1].rearrange("p v o -> p (v o)")
                   .unsqueeze(1).unsqueeze(1).to_broadcast((128, 2, 2, 2)),
                   Hv[:, ::-1, :, h_comp:h_comp + 1]
                   .to_broadcast((128, 2, 2, 2)),
                   Alu.mult)

            merge_cnot_l1h3(0, mA, 0, mB, 0, 0)    # m5 on DVE
            merge_cnot_l1h3(16, mB, 1, mA, 1, 1)   # m6 on Pool

            # RY(m5, b3, th19 = csA col 3) on DVE
            d16 = sb.tile([128, 16], f32, tag="d16")
            nc.vector.tensor_scalar(d16[:], m56[:, 0:16], sA(3), None,
                                    Alu.mult)
            nc.vector.scalar_tensor_tensor(
                m56[:, 0:8], m56[:, 0:8], cA(3), d16[:, 8:16],
                Alu.mult, Alu.subtract)
            nc.vector.scalar_tensor_tensor(
                m56[:, 8:16], m56[:, 8:16], cA(3), d16[:, 0:8],
                Alu.mult, Alu.add)

            # H0 = RY(th20+a)(m6), G = RY(th20-a)(m6)  [a = th23+th26]
            # csA col 7 = (th20+a), col 8 = (th20-a); on Pool
            h0t = sb.tile([128, 16], f32, tag="h0t")
            gt = sb.tile([128, 16], f32, tag="gt")
            da = sb.tile([128, 16], f32, tag="da")
            db = sb.tile([128, 16], f32, tag="db")
            da2 = sb.tile([128, 16], f32, tag="da2")
            db2 = sb.tile([128, 16], f32, tag="db2")
            m6v = m56[:, 16:32]
            nc.gpsimd.tensor_tensor(da[:], m6v,
                                    sA(7).to_broadcast((128, 16)), Alu.mult)
            nc.gpsimd.tensor_tensor(db[:], m6v,
                                    cA(7).to_broadcast((128, 16)), Alu.mult)
            nc.gpsimd.tensor_tensor(h0t[:, 0:8], db[:, 0:8], da[:, 8:16],
                                    Alu.subtract)
            nc.gpsimd.tensor_tensor(h0t[:, 8:16], da[:, 0:8], db[:, 8:16],
                                    Alu.add)
            nc.gpsimd.tensor_tensor(da2[:], m6v,
                                    sA(8).to_broadcast((128, 16)), Alu.mult)
            nc.gpsimd.tensor_tensor(db2[:], m6v,
                                    cA(8).to_broadcast((128, 16)), Alu.mult)
            nc.gpsimd.tensor_tensor(gt[:, 0:8], db2[:, 0:8], da2[:, 8:16],
                                    Alu.subtract)
            nc.gpsimd.tensor_tensor(gt[:, 8:16], da2[:, 0:8], db2[:, 8:16],
                                    Alu.add)

            # m7 = merge: cols l + 16*h; cb = m5 b3 (l in 8-15)
            m7 = sb.tile([128, 256], f32, tag="m7")
            m7v = m7[:].rearrange("p (h l) -> p h l", l=16)
            nc.vector.tensor_tensor(
                m7v[:, :, 0:8],
                m56[:, 0:8].unsqueeze(1).to_broadcast((128, 16, 8)),
                h0t[:].unsqueeze(2).to_broadcast((128, 16, 8)),
                Alu.mult)
            m7v2 = m7[:].rearrange("p (hb hl l) -> p hb hl l", hl=8, l=16)
            nc.gpsimd.tensor_tensor(
                m7v2[:, :, :, 8:16],
                m56[:, 8:16].unsqueeze(1).unsqueeze(1)
                .to_broadcast((128, 2, 8, 8)),
                gt[:].rearrange("p (b x) -> p b x", b=2)[:, ::-1, :]
                .unsqueeze(3).to_broadcast((128, 2, 8, 8)),
                Alu.mult)

            # measurement: z = sum a0*(Rz a0 + s29 a1) + sum a1*(Rzn a1
            # + s29 a0);  a0 = m7[:, 0:128], a1 = m7[:, 128:256]
            w0 = sb.tile([128, 128], f32, tag="w0")
            w1 = sb.tile([128, 128], f32, tag="w1")
            w1b = sb.tile([128, 128], f32, tag="w1b")
            a0 = m7[:, 0:128]
            a1 = m7[:, 128:256]
            nc.vector.tensor_tensor(w0[:], a0, Rz[:], Alu.mult)
            nc.vector.scalar_tensor_tensor(w0[:], a1, nsA[:, 0:1], w0[:],
                                           Alu.mult, Alu.add)
            nc.vector.scalar_tensor_tensor(w0[:], a0, 1.0, w0[:],
                                           Alu.mult, Alu.mult,
                                           accum_out=zac[:, 5:6])
            import concourse.mybir as _mb
            nc.gpsimd.tensor_tensor(w1[:], a1, Rzn[:], Alu.mult)
            nc.gpsimd.tensor_tensor(w1b[:], a0,
                                    nsA[:, 0:1].to_broadcast((128, 128)),
                                    Alu.mult)
            nc.gpsimd.tensor_tensor(w1[:], w1[:], w1b[:], Alu.add)
            nc.gpsimd.tensor_tensor(w1[:], a1, w1[:], Alu.mult)
            nc.vector.tensor_reduce(zac[:, 6:7], w1[:],
                                    _mb.AxisListType.X, Alu.add)
            nc.vector.tensor_tensor(zac[:, 7:8], zac[:, 5:6], zac[:, 6:7],
                                    Alu.add)
            nc.vector.tensor_scalar(out_sb[:, 0:1], zac[:, 7:8], -PI, PI,
                                    Alu.mult, Alu.add)

            nc.sync.dma_start(out_d, out_sb[:])

    nc.compile()
    return nc


def get_nc():
    if "nc" not in _cache:
        _cache["nc"] = _build_program()
    return _cache["nc"]


def kernel(X, e, Ri, Ro, theta):
    from concourse.bass_utils import run_bass_kernel_spmd

    nc = get_nc()
    in_map = {
        "X": np.ascontiguousarray(np.asarray(X, dtype=np.float32)),
        "e": np.ascontiguousarray(np.asarray(e, dtype=np.float32)),
        "Ri": np.ascontiguousarray(np.asarray(Ri, dtype=np.float32)),
        "Ro": np.ascontiguousarray(np.asarray(Ro, dtype=np.float32)),
        "theta": np.ascontiguousarray(np.asarray(theta, dtype=np.float32)),
    }
    res = run_bass_kernel_spmd(
        nc, [dict(in_map) for _ in range(N_CORES)],
        core_ids=list(range(N_CORES)),
    )
    return res.results[0]["out"]


# revision 26
# speedup vs baseline: 1.0717x; 1.0225x over previous
"""Trainium2 Bass kernel for nn_NodeNet (GNN message passing + 15-qubit circuit).

Exact algebraic structure exploited (hand-scheduled version):
1. The 2^15 state stays a tensor product of small components; only the
   q5 component reaches 256 dims and only it depends on the message-passing
   matmuls.  The q10 measurement chain depends on X only, so it runs
   entirely inside the input-DMA window.
2. Final CNOT+RY before each measurement are folded into the observable
   (Heisenberg picture): O = cos(th)*Z_c Z_t + sin(th)*X_t, measured with
   3 fused multiply-accumulate ops instead of gate applications.
3. The last CNOT(3,7)+RY(b7) pair on the q5 chain folds into the m5 x m6
   merge by pre-rotating m6 two ways (theta20+-alpha) - the 256-wide RY
   disappears.
4. Range reduction for sin/cos uses mod: sin(u/2) = Sin(pi - 2pi*frac(
   u/(4pi)+16)), one fused ACT op produces both sin and cos columns.
5. DMA plan minimizes HWDGE serialization (a single device in HW): big
   matrices split between the SP HWDGE queue and the Pool SWDGE queue,
   the last-arriving piece is only 128 columns wide.

Self-contained: hardcodes shapes (N=128, E=1024) and the (pre-simplified)
gate structure.
"""

import math

import numpy as np

N_CORES = 8
PI = math.pi

_cache = {}


def _build_program():
    import concourse.bacc as bacc
    import concourse.mybir as mybir
    import concourse.tile as tile
    from concourse.masks import make_identity

    f32 = mybir.dt.float32
    i32 = mybir.dt.int32
    Alu = mybir.AluOpType
    Act = mybir.ActivationFunctionType

    nc = bacc.Bacc(
        "TRN2",
        target_bir_lowering=False,
        debug=False,
        enable_asserts=False,
        num_devices=1,
    )

    X_d = nc.dram_tensor("X", [128, 5], f32, kind="ExternalInput").ap()
    e_d = nc.dram_tensor("e", [1024], f32, kind="ExternalInput").ap()
    Ri_d = nc.dram_tensor("Ri", [128, 1024], f32, kind="ExternalInput").ap()
    Ro_d = nc.dram_tensor("Ro", [128, 1024], f32, kind="ExternalInput").ap()
    th_d = nc.dram_tensor("theta", [31], f32, kind="ExternalInput").ap()
    out_d = nc.dram_tensor("out", [128, 2], f32, kind="ExternalOutput").ap()

    with tile.TileContext(nc) as tc:
        with (
            tc.tile_pool(name="sbuf", bufs=1) as sb,
            tc.tile_pool(name="psmm", bufs=1, space="PSUM") as psmm,
            tc.tile_pool(name="pstp", bufs=1, space="PSUM") as pstp,
            tc.tile_pool(name="psac", bufs=1, space="PSUM") as psac,
            tc.tile_pool(name="psac2", bufs=1, space="PSUM") as psac2,
        ):
            # ---------------- SBUF tiles ----------------
            Ri_sb = sb.tile([128, 1024], f32, tag="Ri")
            Ro_sb = sb.tile([128, 1024], f32, tag="Ro")
            RiT = sb.tile([128, 1024], f32, tag="RiT")
            RoT = sb.tile([128, 1024], f32, tag="RoT")
            X_sb = sb.tile([128, 5], f32, tag="X")
            TH = sb.tile([128, 31], f32, tag="TH")
            e8_sb = sb.tile([8, 128], f32, tag="e8")
            e_sb = sb.tile([128, 8], f32, tag="e_sb")
            ident = sb.tile([128, 128], f32, tag="ident")

            # ---------------- DMA dispatches ----------------
            # SP HWDGE queue (in order): Ri halves first (earliest big
            # transfers), then theta (broadcast to 128 partitions), e8,
            # and the tail 384 cols of Ro.
            th1 = sb.tile([1, 31], f32, tag="th1")
            nc.sync.dma_start(th1[:], th_d.unsqueeze(0))
            nc.sync.dma_start(Ri_sb[:, 0:512], Ri_d[:, 0:512])
            nc.sync.dma_start(Ri_sb[:, 512:1024], Ri_d[:, 512:1024])
            nc.sync.dma_start(e8_sb[:], e_d.rearrange("(c p) -> c p", c=8))
            nc.sync.dma_start(Ro_sb[:, 640:1024], Ro_d[:, 640:1024])

            # Pool SWDGE queue: X, then Ro cols 0-511, then the small
            # 128-col piece (arrives last; short matmul tail).
            nc.gpsimd.dma_start(X_sb[:], X_d)
            make_identity(nc, ident[:])
            nc.gpsimd.dma_start(Ro_sb[:, 0:512], Ro_d[:, 0:512])
            nc.gpsimd.dma_start(Ro_sb[:, 512:640], Ro_d[:, 512:640])

            # theta broadcast to all partitions via K=1 matmul
            ones1 = sb.tile([1, 128], f32, tag="ones1")
            nc.vector.memset(ones1[:], 1.0)
            th_ps = psac.tile([128, 31], f32, tag="th_ps")
            nc.tensor.matmul(th_ps[:], ones1[:], th1[:], start=True, stop=True)
            nc.vector.tensor_copy(TH[:], th_ps[:])

            # ---------------- constants / warmup ----------------
            # Preload the Sin activation table during the DMA window.
            warm = sb.tile([1, 1], f32, tag="warm")
            nc.vector.memset(warm[:], 0.0)
            nc.scalar.activation(warm[:], warm[:], Act.Sin)

            # sign row sigma[x] = (-1)^x, replicated across partitions
            sigI = sb.tile([128, 128], i32, tag="sigI")
            sigF = sb.tile([128, 128], f32, tag="sigF")
            nc.gpsimd.iota(sigI[:], pattern=[[1, 128]], base=0,
                           channel_multiplier=0)
            nc.vector.tensor_scalar(sigI[:], sigI[:], 1, None, Alu.bitwise_and)
            nc.vector.tensor_copy(sigF[:], sigI[:])
            nc.vector.tensor_scalar(sigF[:], sigF[:], -2.0, 1.0,
                                    Alu.mult, Alu.add)

            # ---------------- angle block A (theta + X qubits) ----------
            # AANG columns (full angles u; we produce cos/sin of u/2):
            #  0: th14   1: th15   2: th16   3: th19   4: th25
            #  5: th17+th21        6: th24+th27
            #  7: th20+th23+th26   8: th20-th23-th26
            #  9: 2*th29          10: 2*th30
            # 11: X0+th10  12: X1+th11  13: X3+th13+th18+th22
            # 14: X4+th14+th19+th28    15: 2*col14
            _hp = tc.high_priority()
            _hp.__enter__()
            AANG = sb.tile([128, 16], f32, tag="AANG")
            scr2 = sb.tile([128, 2], f32, tag="scr2")

            THv = TH[:].rearrange("p (o x) -> p o x", o=1)

            def thcols(lo, n, step=1):
                return THv[:, :, lo:lo + (n - 1) * step + 1:step] if step > 1 \
                    else TH[:, lo:lo + n]

            AAv = AANG[:].rearrange("p (o x) -> p o x", o=1)

            nc.vector.tensor_copy(AANG[:, 0:3], TH[:, 14:17])
            nc.vector.tensor_copy(AAv[:, :, 3:5], thcols(19, 2, 6))
            nc.vector.tensor_tensor(AAv[:, :, 5:7], thcols(17, 2, 7),
                                    thcols(21, 2, 6), Alu.add)
            nc.vector.tensor_tensor(scr2[:, 0:1], TH[:, 23:24], TH[:, 26:27],
                                    Alu.add)
            nc.vector.tensor_tensor(AANG[:, 7:8], TH[:, 20:21], scr2[:, 0:1],
                                    Alu.add)
            nc.vector.tensor_tensor(AANG[:, 8:9], TH[:, 20:21], scr2[:, 0:1],
                                    Alu.subtract)
            nc.vector.tensor_tensor(AANG[:, 9:11], TH[:, 29:31], TH[:, 29:31],
                                    Alu.add)
            nc.vector.tensor_tensor(AANG[:, 11:13], X_sb[:, 0:2], TH[:, 10:12],
                                    Alu.add)
            nc.vector.tensor_tensor(scr2[:], TH[:, 13:15], TH[:, 18:20],
                                    Alu.add)
            nc.vector.tensor_tensor(scr2[:].rearrange("p (o x) -> p o x", o=1),
                                    scr2[:].rearrange("p (o x) -> p o x", o=1),
                                    thcols(22, 2, 6), Alu.add)
            nc.vector.tensor_tensor(AANG[:, 13:15], X_sb[:, 3:5], scr2[:],
                                    Alu.add)
            nc.vector.tensor_tensor(AANG[:, 15:16], AANG[:, 14:15],
                                    AANG[:, 14:15], Alu.add)

            # sincos A: csA[:, j] = sin(u_j/2), csA[:, 16+j] = cos(u_j/2)
            wsA = sb.tile([128, 32], f32, tag="wsA")
            csA = sb.tile([128, 32], f32, tag="csA")
            K4 = 1.0 / (4.0 * PI)
            nc.vector.tensor_scalar(wsA[:, 0:16], AANG[:], K4, 16.0,
                                    Alu.mult, Alu.add)
            nc.vector.tensor_scalar(wsA[:, 16:32], AANG[:], K4, 16.25,
                                    Alu.mult, Alu.add)
            kiA = sb.tile([128, 32], i32, tag="kiA")
            kfA = sb.tile([128, 32], f32, tag="kfA")
            nc.vector.tensor_copy(kiA[:], wsA[:])
            nc.vector.tensor_copy(kfA[:], kiA[:])
            nc.vector.tensor_tensor(wsA[:], wsA[:], kfA[:], Alu.subtract)
            nc.vector.tensor_scalar(kfA[:], wsA[:], 0.5, None, Alu.is_gt)
            nc.vector.scalar_tensor_tensor(wsA[:], kfA[:], -1.0, wsA[:],
                                           Alu.mult, Alu.add)
            nc.scalar.activation(csA[:], wsA[:], Act.Sin, scale=2.0 * PI)

            def sA(j):
                return csA[:, j:j + 1]

            def cA(j):
                return csA[:, 16 + j:16 + j + 1]

            # negated sins for the folded measurements
            nsA = sb.tile([128, 2], f32, tag="nsA")
            nc.vector.tensor_scalar(nsA[:], csA[:, 9:11], -1.0, None, Alu.mult)
            _hp.__exit__(None, None, None)

            # measurement rows (ready early): Rz = cos(th29)*sigma,
            # Rzneg = -Rz
            Rz = sb.tile([128, 128], f32, tag="Rz")
            Rzn = sb.tile([128, 128], f32, tag="Rzn")
            nc.vector.tensor_scalar(Rz[:], sigF[:], cA(9), None, Alu.mult)
            nc.vector.tensor_scalar(Rzn[:], Rz[:], -1.0, None, Alu.mult)

            # ---------------- q10 chain (X-only, hidden under DMA) -------
            # m4 = merge+cnot(q11(L), q10(H)); RY(b1, col5); m8 = merge+
            # cnot(q13(L), m4(H)); RY(b2, col6); measure folded:
            # z10 = c30*cos(q14_full)*<Z_b2> + 2*s30*<X_b2-pair>
            m4 = sb.tile([128, 4], f32, tag="m4")
            m8 = sb.tile([128, 8], f32, tag="m8")
            d4 = sb.tile([128, 4], f32, tag="d4")
            d8 = sb.tile([128, 8], f32, tag="d8")
            zac = sb.tile([128, 8], f32, tag="zac")
            out_sb = sb.tile([128, 2], f32, tag="out")

            csAv = csA[:].rearrange("p (c d b) -> p c d b", d=2, b=8)
            # L=q11 pair: cols (16+12, 12); H=q10 pair: (16+11, 11)
            m4v = m4[:].rearrange("p (t c) -> p t c", c=2)
            # H[t]: t=0 -> col 27, t=1 -> col 11  (start 27 stride -16)
            Hq10 = csA[:].rearrange("p (c x) -> p c x", c=2)[:, ::-1, 11:12]
            Hq10r = csA[:].rearrange("p (c x) -> p c x", c=2)[:, :, 11:12]
            nc.vector.tensor_tensor(
                m4v[:, :, 0:1], Hq10,
                cA(12).unsqueeze(1).to_broadcast((128, 2, 1)), Alu.mult)
            nc.vector.tensor_tensor(
                m4v[:, :, 1:2], Hq10r,
                sA(12).unsqueeze(1).to_broadcast((128, 2, 1)), Alu.mult)
            # RY(m4, b1, col5): a0=m4[:,0:2] (t=0), a1=m4[:,2:4]
            nc.vector.tensor_scalar(d4[:], m4[:], sA(5), None, Alu.mult)
            nc.vector.scalar_tensor_tensor(
                m4[:, 0:2], m4[:, 0:2], cA(5), d4[:, 2:4],
                Alu.mult, Alu.subtract)
            nc.vector.scalar_tensor_tensor(
                m4[:, 2:4], m4[:, 2:4], cA(5), d4[:, 0:2],
                Alu.mult, Alu.add)
            # m8 = merge+cnot(q13(L,b0), m4(H,b1-b2)); ctrl b0, tgt b2
            m8v = m8[:].rearrange("p (h c) -> p h c", c=2)
            m4f = m4[:].rearrange("p (t i) -> p t i", t=2)
            nc.vector.tensor_tensor(
                m8v[:, :, 0:1], m4[:].unsqueeze(2),
                cA(13).unsqueeze(1).to_broadcast((128, 4, 1)), Alu.mult)
            m8v2 = m8[:].rearrange("p (hb1 hb0 c) -> p hb1 hb0 c",
                                   hb0=2, c=2)
            nc.vector.tensor_tensor(
                m8v2[:, :, :, 1:2],
                m4f[:, ::-1, :].unsqueeze(3),
                sA(13).unsqueeze(1).unsqueeze(1)
                .to_broadcast((128, 2, 2, 1)), Alu.mult)
            # RY(m8, b2, col6): a0 = cols 0-3, a1 = cols 4-7
            nc.vector.tensor_scalar(d8[:], m8[:], sA(6), None, Alu.mult)
            nc.vector.scalar_tensor_tensor(
                m8[:, 0:4], m8[:, 0:4], cA(6), d8[:, 4:8],
                Alu.mult, Alu.subtract)
            nc.vector.scalar_tensor_tensor(
                m8[:, 4:8], m8[:, 4:8], cA(6), d8[:, 0:4],
                Alu.mult, Alu.add)
            # measure: w0=sum(a0^2), w1=sum(a1^2), w2=sum(a0*a1)
            nc.vector.scalar_tensor_tensor(d8[:, 0:4], m8[:, 0:4], 1.0,
                                           m8[:, 0:4], Alu.mult, Alu.mult,
                                           accum_out=zac[:, 0:1])
            nc.vector.scalar_tensor_tensor(d8[:, 4:8], m8[:, 4:8], 1.0,
                                           m8[:, 4:8], Alu.mult, Alu.mult,
                                           accum_out=zac[:, 1:2])
            nc.vector.scalar_tensor_tensor(d4[:, 0:4], m8[:, 0:4], 1.0,
                                           m8[:, 4:8], Alu.mult, Alu.mult,
                                           accum_out=zac[:, 2:3])
            # z10 = cA(10)*cA(15)*(w0-w1) - 2*sA(10)*w2
            nc.vector.tensor_tensor(zac[:, 3:4], zac[:, 0:1], zac[:, 1:2],
                                    Alu.subtract)
            nc.vector.tensor_scalar(zac[:, 3:4], zac[:, 3:4], cA(15), cA(10),
                                    Alu.mult, Alu.mult)
            nc.vector.tensor_tensor(zac[:, 4:5], zac[:, 2:3], zac[:, 2:3],
                                    Alu.add)
            nc.vector.scalar_tensor_tensor(
                zac[:, 4:5], zac[:, 4:5], nsA[:, 1:2], zac[:, 3:4],
                Alu.mult, Alu.add)
            nc.vector.tensor_scalar(out_sb[:, 1:2], zac[:, 4:5], -PI, PI,
                                    Alu.mult, Alu.add)

            # ---------------- message passing pipeline ----------------
            # e transpose: [8,128] -> [128,8]
            e_ps = psmm.tile([128, 8], f32, tag="e_ps")
            nc.tensor.transpose(e_ps[:], e8_sb[:], ident[0:8, 0:8])
            nc.scalar.copy(e_sb[:], e_ps[:])

            # bo/bi: bb_ps[:, c*10:+5] = Ro_c^T X ; +5:+10 = Ri_c^T X
            bb_ps = psmm.tile([128, 80], f32, tag="bb")
            for c in range(8):
                nc.tensor.matmul(bb_ps[:, c * 10 + 5:c * 10 + 10],
                                 Ri_sb[:, c * 128:(c + 1) * 128], X_sb[:],
                                 start=True, stop=True)
            for c in range(8):
                nc.tensor.matmul(bb_ps[:, c * 10:c * 10 + 5],
                                 Ro_sb[:, c * 128:(c + 1) * 128], X_sb[:],
                                 start=True, stop=True)

            # transposes: Ri chunks then Ro chunks; copies alternate
            # DVE / ACT / Pool.
            def copy_out(dst, src, eng):
                if eng == 0:
                    nc.vector.tensor_copy(dst, src)
                elif eng == 1:
                    nc.scalar.copy(dst, src)
                else:
                    nc.gpsimd.tensor_copy(dst, src)

            tpri_tiles = {}
            for h in range(2):
                tp = pstp.tile([128, 512], f32, tag=f"tpri{h}")
                tpri_tiles[h] = tp
                for cc in range(4):
                    c = h * 4 + cc
                    nc.tensor.transpose(tp[:, cc * 128:(cc + 1) * 128],
                                        Ri_sb[:, c * 128:(c + 1) * 128],
                                        ident[:])
                copy_out(RiT[:, h * 512:h * 512 + 256], tp[:, 0:256], h)
                copy_out(RiT[:, h * 512 + 256:h * 512 + 512], tp[:, 256:512],
                         1 - h)
            for h in range(2):
                tp = tpri_tiles[0] if h == 1 else \
                    pstp.tile([128, 512], f32, tag="tpro0")
                for cc in range(4):
                    c = h * 4 + cc
                    nc.tensor.transpose(tp[:, cc * 128:(cc + 1) * 128],
                                        Ro_sb[:, c * 128:(c + 1) * 128],
                                        ident[:])
                if h == 0:
                    copy_out(RoT[:, 0:256], tp[:, 0:256], 0)
                    copy_out(RoT[:, 256:512], tp[:, 256:512], 1)
                else:
                    # last piece 512:640 arrives latest - copy its
                    # transpose separately on DVE for the shortest tail
                    copy_out(RoT[:, 640:1024], tp[:, 128:512], 1)
                    copy_out(RoT[:, 512:640], tp[:, 0:128], 0)

            # weight by e: bow[:, c*10:+10] = bb[:, ...] * e_c
            bow = sb.tile([128, 80], f32, tag="bow")
            for g in range(2):
                ev = e_sb[:, g * 4:(g + 1) * 4].rearrange(
                    "p (c o) -> p c o", o=1).to_broadcast((128, 4, 10))
                ov = bow[:, g * 40:(g + 1) * 40].rearrange(
                    "p (c j) -> p c j", j=10)
                iv = bb_ps[:, g * 40:(g + 1) * 40].rearrange(
                    "p (c j) -> p c j", j=10)
                nc.vector.tensor_tensor(ov, iv, ev, Alu.mult)

            # mi/mo accumulation: mm_ps cols 0-4 = mi, 5-9 = mo
            mi_ps = psac.tile([128, 5], f32, tag="mi")
            mo_ps = psac2.tile([128, 5], f32, tag="mo")
            for c in range(8):
                nc.tensor.matmul(mi_ps[:],
                                 RiT[:, c * 128:(c + 1) * 128],
                                 bow[:, c * 10:c * 10 + 5],
                                 start=(c == 0), stop=(c == 7))
                nc.tensor.matmul(mo_ps[:],
                                 RoT[:, c * 128:(c + 1) * 128],
                                 bow[:, c * 10 + 5:c * 10 + 10],
                                 start=(c == 0), stop=(c == 7))

            # ---------------- sincos B (critical path) ----------------
            # angle_q = mm_ps[:, q] + theta_q  (q = 0..7)
            rowB = sb.tile([128, 8], f32, tag="rowB")
            rowB2 = sb.tile([128, 8], f32, tag="rowB2")
            nc.vector.tensor_scalar(rowB[:], TH[:, 0:8], K4, 16.0,
                                    Alu.mult, Alu.add)
            nc.vector.tensor_scalar(rowB2[:], TH[:, 0:8], K4, 16.25,
                                    Alu.mult, Alu.add)

            wsB = sb.tile([128, 16], f32, tag="wsB")
            csB = sb.tile([128, 16], f32, tag="csB")
            nc.vector.scalar_tensor_tensor(wsB[:, 0:5], mi_ps[:], K4,
                                           rowB[:, 0:5], Alu.mult, Alu.add)
            nc.vector.scalar_tensor_tensor(wsB[:, 5:8], mo_ps[:, 0:3], K4,
                                           rowB[:, 5:8], Alu.mult, Alu.add)
            nc.vector.scalar_tensor_tensor(wsB[:, 8:13], mi_ps[:], K4,
                                           rowB2[:, 0:5], Alu.mult, Alu.add)
            nc.vector.scalar_tensor_tensor(wsB[:, 13:16], mo_ps[:, 0:3], K4,
                                           rowB2[:, 5:8], Alu.mult, Alu.add)
            kiB = sb.tile([128, 16], i32, tag="kiB")
            kfB = sb.tile([128, 16], f32, tag="kfB")
            nc.vector.tensor_copy(kiB[:], wsB[:])
            nc.vector.tensor_copy(kfB[:], kiB[:])
            nc.vector.tensor_tensor(wsB[:], wsB[:], kfB[:], Alu.subtract)
            nc.vector.tensor_scalar(kfB[:], wsB[:], 0.5, None, Alu.is_gt)
            nc.vector.scalar_tensor_tensor(wsB[:], kfB[:], -1.0, wsB[:],
                                           Alu.mult, Alu.add)
            nc.scalar.activation(csB[:], wsB[:], Act.Sin, scale=2.0 * PI)
            # csB: sin(q) at col q, cos(q) at col 8+q

            # ---------------- q5 evolution ----------------
            # Level 0 (batched): mA = {m0=(q0,q1), m2=(q4,q5)},
            #                    mB = {m1=(q3,q2), m3=(q7,q6)}
            # layout: col = comp + 2*b0 + 4*b1   (b0 = L bit, b1 = H bit)
            mA = sb.tile([128, 8], f32, tag="mA")
            mB = sb.tile([128, 8], f32, tag="mB")
            csBv = csB[:].rearrange("p (c d b) -> p c d b", d=2, b=4)

            def level0(mt, lq, hq, eng):
                # L pair cols (8+lq, lq); H pair (8+hq, hq); comps lq,lq+4
                tt = nc.vector.tensor_tensor if eng == 0 else \
                    nc.gpsimd.tensor_tensor
                ov = mt[:].rearrange("p (b1 b0 c) -> p b1 b0 c", b0=2, c=2)
                # operand views: col(H) = 8 - 8*b1 + 4*comp + hq
                Hb = csB[:].rearrange("p (c d b) -> p c d b", d=2, b=4)[
                    :, ::-1, :, hq:hq + 1].rearrange("p c d o -> p c (d o)")
                Hbr = csB[:].rearrange("p (c d b) -> p c d b", d=2, b=4)[
                    :, :, :, hq:hq + 1].rearrange("p c d o -> p c (d o)")
                Lcb = csBv[:, 1, :, lq:lq + 1].rearrange("p d o -> p (d o)")\
                    .unsqueeze(1).to_broadcast((128, 2, 2))
                Lsb = csBv[:, 0, :, lq:lq + 1].rearrange("p d o -> p (d o)")\
                    .unsqueeze(1).to_broadcast((128, 2, 2))
                tt(ov[:, :, 0, :], Lcb, Hb, Alu.mult)
                tt(ov[:, :, 1, :], Lsb, Hbr, Alu.mult)

            level0(mA, 0, 1, 0)   # m0=(q0 ctrl, q1 tgt), m2=(q4, q5) on DVE
            level0(mB, 3, 2, 1)   # m1=(q3, q2), m3=(q7, q6) on Pool

            # b1 RYs: mA comps (m0: th15, m2: th14) -> csA cols (1, 0)
            #         mB comps (m1: th16, m3: th15) -> csA cols (2, 1)
            sc4 = sb.tile([128, 4], f32, tag="sc4")
            sc4b = sb.tile([128, 4], f32, tag="sc4b")
            sc4c = sb.tile([128, 4], f32, tag="sc4c")
            sc4d = sb.tile([128, 4], f32, tag="sc4d")
            dA = sb.tile([128, 8], f32, tag="dA")
            dB = sb.tile([128, 8], f32, tag="dB")

            def ry_b1_batch(mt, c_hi, scv, scv2, dt, eng):
                # coefs: comp0 at csA col c_hi, comp1 at col c_hi-1
                tt = nc.vector.tensor_tensor if eng == 0 else \
                    nc.gpsimd.tensor_tensor
                cview = csA[:].rearrange("p (o x) -> p o x", o=1)[
                    :, :, 16 + c_hi - 1:16 + c_hi + 1][:, :, ::-1]
                sview = csA[:].rearrange("p (o x) -> p o x", o=1)[
                    :, :, c_hi - 1:c_hi + 1][:, :, ::-1]
                cb = cview.to_broadcast((128, 2, 2))
                sb_ = sview.unsqueeze(1).to_broadcast((128, 2, 2, 2))
                a0 = mt[:, 0:4].rearrange("p (b0 c) -> p b0 c", c=2)
                a1 = mt[:, 4:8].rearrange("p (b0 c) -> p b0 c", c=2)
                dv = dt[:].rearrange("p (b1 b0 c) -> p b1 b0 c", b0=2, c=2)
                t0 = scv[:].rearrange("p (b0 c) -> p b0 c", c=2)
                t1 = scv2[:].rearrange("p (b0 c) -> p b0 c", c=2)
                tt(t0, a0, cb, Alu.mult)
                tt(t1, a1, cb, Alu.mult)
                tt(dv, mt[:].rearrange("p (b1 b0 c) -> p b1 b0 c", b0=2, c=2),
                   sb_, Alu.mult)
                tt(a0, t0, dv[:, 1], Alu.subtract)
                tt(a1, t1, dv[:, 0], Alu.add)

            ry_b1_batch(mA, 1, sc4, sc4b, dA, 0)
            ry_b1_batch(mB, 2, sc4c, sc4d, dB, 1)

            # b0 RY on m0 (th25 = csA col 4): m0 = mA comp 0, strided
            m0v = mA[:].rearrange("p (b1 b0 c) -> p b1 b0 c", b0=2, c=2)
            nc.vector.tensor_scalar(
                dA[:].rearrange("p (b1 b0 c) -> p b1 b0 c", b0=2, c=2)
                [:, :, :, 0:1],
                m0v[:, :, :, 0:1], sA(4), None, Alu.mult)
            dAv = dA[:].rearrange("p (b1 b0 c) -> p b1 b0 c", b0=2, c=2)
            nc.vector.scalar_tensor_tensor(
                m0v[:, :, 0, 0:1], m0v[:, :, 0, 0:1], cA(4),
                dAv[:, :, 1, 0:1], Alu.mult, Alu.subtract)
            nc.vector.scalar_tensor_tensor(
                m0v[:, :, 1, 0:1], m0v[:, :, 1, 0:1], cA(4),
                dAv[:, :, 0, 0:1], Alu.mult, Alu.add)

            # m5 = merge+cnot(m0, m1; ctrl=b1 of m0, tgt=b1 of m1) on DVE
            # m6 = merge+cnot(m3, m2; same) on Pool
            # m56: m5 = cols 0-15, m6 = 16-31; col = l + 4*h
            m56 = sb.tile([128, 32], f32, tag="m56")

            def merge_cnot_l1h3(dst_off, Lt, l_comp, Ht, h_comp, eng):
                tt = nc.vector.tensor_tensor if eng == 0 else \
                    nc.gpsimd.tensor_tensor
                # L[cb, v0] at col l_comp + 2*v0 + 4*cb
                Lv = Lt[:].rearrange("p (cb v0 c) -> p cb v0 c", v0=2, c=2)
                Hv = Ht[:].rearrange("p (hb1 hb0 c) -> p hb1 hb0 c",
                                     hb0=2, c=2)
                ov = m56[:, dst_off:dst_off + 16].rearrange(
                    "p (hb1 hb0 cb v0) -> p hb1 hb0 cb v0", hb0=2, cb=2, v0=2)
                # cb=0: out = L[0, v0] * H[hb1, hb0]
                tt(ov[:, :, :, 0, :],
                   Lv[:, 0, :, l_comp:l_comp + 1].rearrange("p v o -> p (v o)")
                   .unsqueeze(1).unsqueeze(1).to_broadcast((128, 2, 2, 2)),
                   Hv[:, :, :, h_comp:h_comp + 1]
                   .to_broadcast((128, 2, 2, 2)),
                   Alu.mult)
                # cb=1: out = L[1, v0] * H[1-hb1, hb0]
                tt(ov[:, :, :, 1, :],
                   Lv[:, 1, :, l_comp:l_comp + 1].rearrange("p v o -> p (v o)")
                   .unsqueeze(1).unsqueeze(1).to_broadcast((128, 2, 2, 2)),
                   Hv[:, ::-1, :, h_comp:h_comp + 1]
                   .to_broadcast((128, 2, 2, 2)),
                   Alu.mult)

            merge_cnot_l1h3(0, mA, 0, mB, 0, 0)    # m5 on DVE
            merge_cnot_l1h3(16, mB, 1, mA, 1, 1)   # m6 on Pool

            # RY(m5, b3, th19 = csA col 3) on DVE
            d16 = sb.tile([128, 16], f32, tag="d16")
            nc.vector.tensor_scalar(d16[:], m56[:, 0:16], sA(3), None,
                                    Alu.mult)
            nc.vector.scalar_tensor_tensor(
                m56[:, 0:8], m56[:, 0:8], cA(3), d16[:, 8:16],
                Alu.mult, Alu.subtract)
            nc.vector.scalar_tensor_tensor(
                m56[:, 8:16], m56[:, 8:16], cA(3), d16[:, 0:8],
                Alu.mult, Alu.add)

            # H0 = RY(th20+a)(m6), G = RY(th20-a)(m6)  [a = th23+th26]
            # csA col 7 = (th20+a), col 8 = (th20-a); on Pool
            h0t = sb.tile([128, 16], f32, tag="h0t")
            gt = sb.tile([128, 16], f32, tag="gt")
            da = sb.tile([128, 16], f32, tag="da")
            db = sb.tile([128, 16], f32, tag="db")
            da2 = sb.tile([128, 16], f32, tag="da2")
            db2 = sb.tile([128, 16], f32, tag="db2")
            m6v = m56[:, 16:32]
            nc.gpsimd.tensor_tensor(da[:], m6v,
                                    sA(7).to_broadcast((128, 16)), Alu.mult)
            nc.gpsimd.tensor_tensor(db[:], m6v,
                                    cA(7).to_broadcast((128, 16)), Alu.mult)
            nc.gpsimd.tensor_tensor(h0t[:, 0:8], db[:, 0:8], da[:, 8:16],
                                    Alu.subtract)
            nc.gpsimd.tensor_tensor(h0t[:, 8:16], da[:, 0:8], db[:, 8:16],
                                    Alu.add)
            nc.gpsimd.tensor_tensor(da2[:], m6v,
                                    sA(8).to_broadcast((128, 16)), Alu.mult)
            nc.gpsimd.tensor_tensor(db2[:], m6v,
                                    cA(8).to_broadcast((128, 16)), Alu.mult)
            nc.gpsimd.tensor_tensor(gt[:, 0:8], db2[:, 0:8], da2[:, 8:16],
                                    Alu.subtract)
            nc.gpsimd.tensor_tensor(gt[:, 8:16], da2[:, 0:8], db2[:, 8:16],
                                    Alu.add)

            # m7 = merge: cols l + 16*h; cb = m5 b3 (l in 8-15)
            m7 = sb.tile([128, 256], f32, tag="m7")
            m7v = m7[:].rearrange("p (h l) -> p h l", l=16)
            nc.vector.tensor_tensor(
                m7v[:, :, 0:8],
                m56[:, 0:8].unsqueeze(1).to_broadcast((128, 16, 8)),
                h0t[:].unsqueeze(2).to_broadcast((128, 16, 8)),
                Alu.mult)
            m7v2 = m7[:].rearrange("p (hb hl l) -> p hb hl l", hl=8, l=16)
            nc.gpsimd.tensor_tensor(
                m7v2[:, :, :, 8:16],
                m56[:, 8:16].unsqueeze(1).unsqueeze(1)
                .to_broadcast((128, 2, 8, 8)),
                gt[:].rearrange("p (b x) -> p b x", b=2)[:, ::-1, :]
                .unsqueeze(3).to_broadcast((128, 2, 8, 8)),
                Alu.mult)

            # measurement: z = sum a0*(Rz a0 + s29 a1) + sum a1*(Rzn a1
            # + s29 a0);  a0 = m7[:, 0:128], a1 = m7[:, 128:256]
            w0 = sb.tile([128, 128], f32, tag="w0")
            w1 = sb.tile([128, 128], f32, tag="w1")
            w1b = sb.tile([128, 128], f32, tag="w1b")
            a0 = m7[:, 0:128]
            a1 = m7[:, 128:256]
            nc.vector.tensor_tensor(w0[:], a0, Rz[:], Alu.mult)
            nc.vector.scalar_tensor_tensor(w0[:], a1, nsA[:, 0:1], w0[:],
                                           Alu.mult, Alu.add)
            nc.vector.scalar_tensor_tensor(w0[:], a0, 1.0, w0[:],
                                           Alu.mult, Alu.mult,
                                           accum_out=zac[:, 5:6])
            import concourse.mybir as _mb
            nc.gpsimd.tensor_tensor(w1[:], a1, Rzn[:], Alu.mult)
            nc.gpsimd.tensor_tensor(w1b[:], a0,
                                    nsA[:, 0:1].to_broadcast((128, 128)),
                                    Alu.mult)
            nc.gpsimd.tensor_tensor(w1[:], w1[:], w1b[:], Alu.add)
            nc.gpsimd.tensor_tensor(w1[:], a1, w1[:], Alu.mult)
            nc.vector.tensor_reduce(zac[:, 6:7], w1[:],
                                    _mb.AxisListType.X, Alu.add)
            nc.vector.tensor_tensor(zac[:, 7:8], zac[:, 5:6], zac[:, 6:7],
                                    Alu.add)
            nc.vector.tensor_scalar(out_sb[:, 0:1], zac[:, 7:8], -PI, PI,
                                    Alu.mult, Alu.add)

            nc.sync.dma_start(out_d, out_sb[:])

    nc.compile()
    return nc


def get_nc():
    if "nc" not in _cache:
        _cache["nc"] = _build_program()
    return _cache["nc"]


def kernel(X, e, Ri, Ro, theta):
    from concourse.bass_utils import run_bass_kernel_spmd

    nc = get_nc()
    in_map = {
        "X": np.ascontiguousarray(np.asarray(X, dtype=np.float32)),
        "e": np.ascontiguousarray(np.asarray(e, dtype=np.float32)),
        "Ri": np.ascontiguousarray(np.asarray(Ri, dtype=np.float32)),
        "Ro": np.ascontiguousarray(np.asarray(Ro, dtype=np.float32)),
        "theta": np.ascontiguousarray(np.asarray(theta, dtype=np.float32)),
    }
    res = run_bass_kernel_spmd(
        nc, [dict(in_map) for _ in range(N_CORES)],
        core_ids=list(range(N_CORES)),
    )
    return res.results[0]["out"]
